# revision 32
# baseline (speedup 1.0000x reference)
"""Trainium2 Bass kernel for nn_AttnBYOL (Performer linear-attention BYOL net).

Self-contained: takes FULL inputs, shards batch B=32 across 8 NeuronCores
(4 batches/core), runs one SPMD Bass/Tile program, gathers full output.

v3 design notes:
- No max subtraction anywhere: pq = exp(x@proj) raw (range-checked on real
  data), pk = exp(x@proj - diag).  The reference's q-side row-max and +eps
  are reproduced exactly through a rank-1 correction A += epsn x G with
  epsn[n] = eps*maxrow(pk)[n]*e^{2*diag[n]}; the k-side +eps enters as a
  per-partition scalar add (eps*e^M*Vsum) folded into the ctx PSUM->SBUF
  copy.  The rank-1 rides as row 115 of pq^T (epsn, via a DRAM-bounced
  DMA) against row 115 of the j-major ctx block (G, via an extra ctx_sb
  column through the transpose), so pass 3 is two matmuls per chunk.
- Activations keep a padded bf16 layout with 128-wide head slots so every
  head transpose is a single [128,128] DMA-transpose (XBAR) instead of a
  PE matmul + PSUM copy; ones columns at slot offset 81 give v_aug/ksum
  for free.
- ctx accumulates e-major ([82, 244]) with v_aug as the 82-wide stationary;
  pq^T comes from a j-major projection matmul (projection stationary).
- LayerNorm: one 3-group bn_stats per chunk + per-chunk bn_aggr; the
  attention diag comes from closed-form even/odd group algebra (no
  per-head bn_aggr).  bf16 copies are 4-chunk batched 4D-AP copies.
"""
import os
import numpy as np
import ml_dtypes
from contextlib import ExitStack

import concourse.bass as bass
import concourse.tile as tile
from concourse import bacc, mybir, masks, bass_isa
from concourse.bass_utils import run_bass_kernel_spmd

FP = mybir.dt.float32
BF = mybir.dt.bfloat16
AX = mybir.AxisListType
ALU = mybir.AluOpType
ACTF = mybir.ActivationFunctionType

B, L, H, F, D = 32, 1024, 3, 81, 243
NF, FFH = 243, 972
NE, ND = 2, 2
NCORES = 8
NB = B // NCORES          # batches per core
NT = NB * L               # tokens per core (4096)
NCH = NT // 128           # 32 token chunks
CPB = L // 128            # 8 chunks per batch
DN = float(F) ** -0.25    # 1/3
DSCALE = 0.5 * DN * DN    # 1/18
EPS = 1e-4
LOG_EPS = float(np.log(EPS))
LNEPS = 1e-5
XS = 384                  # bf16 padded chunk stride (3 x 128 head slots)
HS = 128                  # head slot stride (81 data + ones col + pad)
KS = 256                  # pk chunk stride (243 + ones col + pad)
YS = 243                  # fp32 activation chunk stride
NEC = (F + 1) // 2        # bn_stats even-group count (41)
NOC = F // 2              # bn_stats odd-group count (40)

_cache = {}


def _build(ln_trivial: bool):
    nc = bacc.Bacc("TRN2", target_bir_lowering=False, debug=False,
                   enable_asserts=False, num_devices=NCORES)

    d_xin = nc.dram_tensor("xin", [NT, D], FP, kind="ExternalInput").ap()
    d_xout = nc.dram_tensor("xout", [NT, D], FP, kind="ExternalInput").ap()
    d_projt = nc.dram_tensor("projt", [6, 84, NF], BF, kind="ExternalInput").ap()
    d_w1 = nc.dram_tensor("w1", [4, D, FFH], BF, kind="ExternalInput").ap()
    d_w2e = nc.dram_tensor("w2e", [4, 993, D], BF, kind="ExternalInput").ap()
    d_b1c = nc.dram_tensor("b1c", [4, 128, 8], FP, kind="ExternalInput").ap()
    d_lnw = nc.dram_tensor("lnw", [128, D], FP, kind="ExternalInput").ap()
    d_lnb = nc.dram_tensor("lnb", [128, D], FP, kind="ExternalInput").ap()
    d_out = nc.dram_tensor("out", [NT, D], FP, kind="ExternalOutput").ap()

    with TileKernel(nc, ln_trivial) as k:
        k.run(d_xin, d_xout, d_projt, d_w1, d_w2e, d_b1c, d_lnw, d_lnb, d_out)

    nc.compile()
    return nc


class TileKernel:
    def __init__(self, nc, ln_trivial):
        self.nc = nc
        self.ln_trivial = ln_trivial
        self.ctx = ExitStack()

    def __enter__(self):
        self.tc = self.ctx.enter_context(tile.TileContext(self.nc))
        return self

    def __exit__(self, *a):
        return self.ctx.__exit__(*a)

    def pool(self, name, bufs, space="SBUF"):
        return self.ctx.enter_context(
            self.tc.tile_pool(name=name, bufs=bufs, space=space))

    # ================= top level =================
    def run(self, d_xin, d_xout, d_projt, d_w1, d_w2e, d_b1c, d_lnw, d_lnb,
            d_out):
        nc = self.nc

        const = self.pool("const", 1)
        wpool = self.pool("wts", 1)
        resid = self.pool("resid", 2)           # fp32 [128, 32*243] streams
        xbf = self.pool("xbf", 1)               # padded bf16 [128, 32*384]
        xt = self.pool("xt", 1)                 # transposed activations
        lth_p = self.pool("lth", 1)             # persistent l_in^T
        pqp = self.pool("pq", 1)                # pq^T j-major
        pkp = self.pool("pk", 1)                # pk token-major padded
        gel = self.pool("gelu", 10)
        st = self.pool("st", 3)                 # small stats tiles
        nbp = self.pool("nb", 2)                # -diag per head
        nbl = self.pool("nbL", 1)               # -diag of l_in final
        cxs = self.pool("cxs", 2)               # ctx_sb/ctxj/eps_sb

        ident = const.tile([128, 128], BF)
        masks.make_identity(nc, ident[:])
        zero_c = const.tile([128, 1], FP)
        nc.vector.memset(zero_c[:], 0.0)
        lneps_c = const.tile([128, 1], FP)
        nc.vector.memset(lneps_c[:], LNEPS)
        logeps_c = const.tile([128, 1], FP)
        nc.vector.memset(logeps_c[:], LOG_EPS)
        self.C = dict(ident=ident, zero=zero_c, lneps=lneps_c, logeps=logeps_c)
        if not self.ln_trivial:
            lnw_t = const.tile([128, D], FP)
            lnb_t = const.tile([128, D], FP)
            nc.sync.dma_start(out=lnw_t[:], in_=d_lnw)
            nc.sync.dma_start(out=lnb_t[:], in_=d_lnb)
            self.C["lnw"] = lnw_t
            self.C["lnb"] = lnb_t
        projt_t = []
        for a in range(6):
            t = const.tile([F, NF], BF, tag=f"projt{a}", name=f"projt{a}")
            nc.sync.dma_start(out=t[:], in_=d_projt[a])
            projt_t.append(t)

        self.P = dict(resid=resid, xbf=xbf, xt=xt, lth=lth_p, pq=pqp, pk=pkp,
                      gel=gel, st=st, nb=nbp, nbl=nbl, cxs=cxs, w=wpool)

        def load_ff_w(i):
            w1h = []
            for h in range(H):
                t = wpool.tile([F, FFH], BF, tag=f"w1h{h}", name=f"w1h{h}")
                nc.sync.dma_start(out=t[:], in_=d_w1[i, h * F:(h + 1) * F])
                w1h.append(t)
            w2k = []
            for kk in range(8):
                kw = 128 if kk < 7 else 97
                t = wpool.tile([kw, D], BF, tag=f"w2k{kk}", name=f"w2k{kk}")
                nc.sync.dma_start(out=t[:], in_=d_w2e[i, kk * 128: kk * 128 + kw])
                w2k.append(t)
            b1c = wpool.tile([128, 8], FP, tag="b1c", name="b1c")
            nc.sync.dma_start(out=b1c[:], in_=d_b1c[i])
            return w1h, w2k, b1c

        # ---------------- encoder ----------------
        X = resid.tile([128, NCH * YS], FP, tag="resid", name="resid")
        nc.sync.dma_start(out=X[:].rearrange("p (c d) -> p c d", d=YS),
                          in_=d_xin.rearrange("(c p) d -> p c d", p=128))
        Xb = self.make_bf16(X)
        nbh = self.diag_stats(X, self.P["nb"], raw=True)

        for i in range(NE):
            xth = self.transpose_heads(Xb)
            Y = resid.tile([128, NCH * YS], FP, tag="resid", name="resid")
            self.attention(projt_t[i], xth, nbh, Xb, X, Y)
            Yb = self.layer_norm(Y, bf16_out=True)[0]
            w1h, w2k, b1c = load_ff_w(i)
            X2 = resid.tile([128, NCH * YS], FP, tag="resid", name="resid")
            self.ff(Yb, Y, X2, w1h, w2k, b1c)
            last = (i == NE - 1)
            Xb, nbh = self.layer_norm(X2, bf16_out=True,
                                      diag_pool=(self.P["nbl"] if last
                                                 else self.P["nb"]))
            X = X2

        # persistent transposed l_in for the decoder cross attentions
        lth = [self.P["lth"].tile([F, NT], BF, tag=f"lth{h}", name=f"lth{h}")
               for h in range(H)]
        self.transpose_heads(Xb, dst=lth)
        nbhL = nbh

        # ---------------- decoder ----------------
        X = resid.tile([128, NCH * YS], FP, tag="resid", name="resid")
        nc.sync.dma_start(out=X[:].rearrange("p (c d) -> p c d", d=YS),
                          in_=d_xout.rearrange("(c p) d -> p c d", p=128))
        Xb = self.make_bf16(X)
        nbh = self.diag_stats(X, self.P["nb"], raw=True)

        for i in range(ND):
            xth = self.transpose_heads(Xb)
            Y = resid.tile([128, NCH * YS], FP, tag="resid", name="resid")
            self.attention(projt_t[2 + 2 * i], xth, nbh, Xb, X, Y)
            A1b = self.layer_norm(Y, bf16_out=True)[0]          # a1
            Y2 = resid.tile([128, NCH * YS], FP, tag="resid", name="resid")
            self.attention(projt_t[3 + 2 * i], lth, nbhL, A1b, Y, Y2)
            A2b = self.layer_norm(Y2, bf16_out=True)[0]         # a2
            w1h, w2k, b1c = load_ff_w(2 + i)
            X2 = resid.tile([128, NCH * YS], FP, tag="resid", name="resid")
            self.ff(A2b, Y2, X2, w1h, w2k, b1c)
            last = (i == ND - 1)
            if last:
                self.layer_norm(X2, bf16_out=False)
            else:
                Xb, nbh = self.layer_norm(X2, bf16_out=True,
                                          diag_pool=self.P["nb"])
            X = X2

        nc.sync.dma_start(out=d_out.rearrange("(c p) d -> p c d", p=128),
                          in_=X[:].rearrange("p (c d) -> p c d", d=YS))

    # ================= building blocks =================
    def _pad_out_view(self, Xb, g):
        """4-chunk padded output view: [128, 4, 3, 81] at chunk group g."""
        return Xb[:, g * 4 * XS:(g + 1) * 4 * XS].rearrange(
            "p (c h y) -> p c h y", h=H, y=HS)[:, :, :, 0:F]

    def _flat_in_view(self, X, g):
        """4-chunk fp32 input view: [128, 4, 3, 81] at chunk group g."""
        return X[:, g * 4 * YS:(g + 1) * 4 * YS].rearrange(
            "p (c h f) -> p c h f", h=H, f=F)

    def make_bf16(self, X, Xb=None):
        """fp32 token-major -> padded bf16 (128-stride head slots, ones)."""
        nc = self.nc
        if Xb is None:
            Xb = self.P["xbf"].tile([128, NCH * XS], BF, tag="xbf", name="xbf")
        v3 = Xb[:].rearrange("p (c x) -> p c x", x=HS)
        nc.vector.memset(v3[:, :, F], 1.0)
        for g in range(NCH // 4):
            nc.any.tensor_copy(self._pad_out_view(Xb, g),
                               self._flat_in_view(X, g))
        return Xb

    def diag_stats(self, X, pool, raw, sw=None, mvg=None, rs2=None):
        """nbh[h] = -DSCALE * ||x_h||^2 (raw) or of the LN output (from
        even/odd bn_stats group algebra).  X used only when sw is None."""
        nc, st = self.nc, self.P["st"]
        if sw is None:
            sw = st.tile([128, 18 * NCH], FP, tag="sw", name="sw")
            for c in range(NCH):
                for h in range(H):
                    nc.vector.bn_stats(
                        out=sw[:, c * 18 + 6 * h: c * 18 + 6 * h + 6],
                        in_=X[:, c * YS + h * F: c * YS + (h + 1) * F])
        swv = sw[:].rearrange("p (c s) -> p c s", s=18)
        mu81 = None
        if not raw:
            mvv = mvg[:].rearrange("p (c t) -> p c t", t=2)
            mu = mvv[:, :, 0]
            mu81 = st.tile([128, NCH], FP, tag="mu81", name="mu81")
            nc.vector.tensor_scalar_mul(mu81[:], mu, float(D))
        nbh = [pool.tile([128, NCH], FP, tag=f"nbh{h}", name=f"nbh{h}")
               for h in range(H)]
        for h in range(H):
            me, m2e = swv[:, :, 6 * h + 1], swv[:, :, 6 * h + 2]
            mo, m2o = swv[:, :, 6 * h + 4], swv[:, :, 6 * h + 5]
            a = st.tile([128, NCH], FP, tag="dga", name="dga")
            q = st.tile([128, NCH], FP, tag="dgq", name="dgq")
            nc.vector.tensor_mul(a[:], me, me)
            nc.vector.scalar_tensor_tensor(out=q[:], in0=a[:], scalar=float(NEC),
                                           in1=m2e, op0=ALU.mult, op1=ALU.add)
            nc.vector.tensor_mul(a[:], mo, mo)
            nc.vector.scalar_tensor_tensor(out=a[:], in0=a[:], scalar=float(NOC),
                                           in1=m2o, op0=ALU.mult, op1=ALU.add)
            nc.vector.tensor_add(q[:], q[:], a[:])      # sum x^2 over head
            if raw:
                nc.vector.tensor_scalar_mul(nbh[h][:], q[:], -DSCALE)
                continue
            # s = 41*me + 40*mo ; u = (q + mu*(D*mu - 2*s)) * rs^2
            s = st.tile([128, NCH], FP, tag="dgs", name="dgs")
            nc.vector.tensor_scalar_mul(s[:], mo, float(NOC))
            nc.vector.scalar_tensor_tensor(out=s[:], in0=me, scalar=float(NEC),
                                           in1=s[:], op0=ALU.mult, op1=ALU.add)
            nc.vector.scalar_tensor_tensor(out=s[:], in0=s[:], scalar=-2.0,
                                           in1=mu81[:], op0=ALU.mult,
                                           op1=ALU.add)
            mvv = mvg[:].rearrange("p (c t) -> p c t", t=2)
            nc.vector.tensor_mul(s[:], s[:], mvv[:, :, 0])
            nc.vector.tensor_add(s[:], s[:], q[:])
            nc.vector.tensor_mul(s[:], s[:], rs2[:])
            nc.vector.tensor_scalar_mul(nbh[h][:], s[:], -DSCALE)
        return nbh

    def transpose_heads(self, Xb, dst=None):
        """padded token-major -> per-head feature-major via PE transposes."""
        nc = self.nc
        ident = self.C["ident"]
        if dst is None:
            dst = [self.P["xt"].tile([F, NT], BF, tag=f"xt{h}",
                                     name=f"xth{h}")
                   for h in range(H)]
        with ExitStack() as mctx:
            tp = mctx.enter_context(
                self.tc.tile_pool(name="tph", bufs=3, space="PSUM"))
            for c in range(NCH):
                for h in range(H):
                    ps = tp.tile([F, 128], BF, tag="tph", name="tph")
                    nc.tensor.transpose(
                        ps[0:F, 0:128],
                        Xb[:, c * XS + h * HS: c * XS + h * HS + F],
                        ident[:, :])
                    nc.any.tensor_copy(dst[h][:, c * 128:(c + 1) * 128],
                                       ps[0:F, 0:128])
        return dst

    def attention(self, projt, xth, nbh, Vb, Xres, Y):
        """Y = performer_attention(q=k=xth-owner, v=Vb) + Xres.

        Batch-outer loop: each batch finishes all 3 heads before the next
        starts, so the consumer LayerNorm's per-chunk stats can overlap the
        remaining batches (keeps PE fed and the HAM warm)."""
        nc = self.nc
        ident = self.C["ident"]
        st, cxs = self.P["st"], self.P["cxs"]
        with ExitStack() as ps_ctx:
            zdp = ps_ctx.enter_context(
                self.tc.tile_pool(name="zdp", bufs=3, space="PSUM"))
            tpp = ps_ctx.enter_context(
                self.tc.tile_pool(name="tpp", bufs=1, space="PSUM"))
            ctxp = ps_ctx.enter_context(
                self.tc.tile_pool(name="ctxp", bufs=2, space="PSUM"))
            app = ps_ctx.enter_context(
                self.tc.tile_pool(name="app", bufs=2, space="PSUM"))
            drp = ps_ctx.enter_context(
                self.tc.tile_pool(name="drp", bufs=2, space="DRAM"))
            for b in range(NB):
                tb = b * L           # first token of the batch
                for h in range(H):
                    # ---- pq^T = exp(projt^T @ x^T), j-major, no bias ----
                    # pqT1 row 115 carries epsn (q-side eps rank-1 factor)
                    pqT0 = self.P["pq"].tile([128, L], BF, tag="pqT0",
                                             name="pqT0")
                    pqT1 = self.P["pq"].tile([116, L], BF, tag="pqT1",
                                             name="pqT1")
                    for jb, jw, pq in ((0, 128, pqT0), (1, 115, pqT1)):
                        for t in range(L // 512):
                            zt = zdp.tile([128, 512], FP, tag="zd", name="zd")
                            nc.tensor.matmul(
                                zt[0:jw, :],
                                projt[:, jb * 128: jb * 128 + jw],
                                xth[h][:, tb + t * 512: tb + (t + 1) * 512],
                                start=True, stop=True)
                            nc.scalar.activation(
                                pq[0:jw, t * 512:(t + 1) * 512],
                                zt[0:jw, :], ACTF.Exp,
                                bias=self.C["zero"][0:jw, :])
                    # ---- pk = exp(zd - diag), token-major ----
                    pk = self.P["pk"].tile([128, CPB * KS], BF, tag="pk",
                                           name="pk")
                    nc.vector.memset(
                        pk[:].rearrange("p (c x) -> p c x", x=KS)[:, :, NF],
                        1.0)
                    for cc in range(CPB):
                        c = b * CPB + cc
                        zk = zdp.tile([128, NF], FP, tag="zd", name="zd")
                        nc.tensor.matmul(zk[:],
                                         xth[h][:, c * 128:(c + 1) * 128],
                                         projt[0:F, :], start=True, stop=True)
                        nc.scalar.activation(pk[:, cc * KS: cc * KS + NF],
                                             zk[:], ACTF.Exp,
                                             bias=nbh[h][:, c:c + 1])
                    nbs = nbh[h][:, b * CPB:(b + 1) * CPB]
                    mx = st.tile([128, CPB], FP, tag="mx", name="mx")
                    nc.vector.tensor_reduce(
                        mx[:],
                        pk[:].rearrange("p (c x) -> p c x", x=KS)[:, :, 0:NF],
                        axis=AX.X, op=ALU.max)
                    # ---- epsn = eps*mx*e^{2 diag} ; emx = mx*e^{diag} ----
                    e2d = st.tile([128, CPB], FP, tag="e2d", name="e2d")
                    nc.scalar.activation(e2d[:], nbs, ACTF.Exp,
                                         scale=-2.0, bias=self.C["logeps"][:])
                    epsn = st.tile([128, CPB], BF, tag="epsn", name="epsn")
                    nc.vector.tensor_mul(epsn[:], mx[:], e2d[:])
                    ed = st.tile([128, CPB], FP, tag="ed", name="ed")
                    nc.scalar.activation(ed[:], nbs, ACTF.Exp, scale=-1.0,
                                         bias=self.C["zero"][:])
                    emx = st.tile([128, CPB], FP, tag="emx", name="emx")
                    nc.vector.tensor_mul(emx[:], mx[:], ed[:])
                    # epsn -> pqT1 row 115 (transpose + DRAM bounce on the
                    # gpsimd SWDGE queue; matmul stationaries must start at
                    # partition 0/32/64 so a row tile is required)
                    tr = tpp.tile([CPB, 128], BF, tag="tp", name="tp")
                    nc.tensor.transpose(tr[0:CPB, 0:128], epsn[:, :],
                                        ident[:, :])
                    eps_sb = cxs.tile([CPB, 128], BF, tag="eps_sb",
                                      name="eps_sb")
                    nc.any.tensor_copy(eps_sb[:], tr[0:CPB, 0:128])
                    eps_d = drp.tile([CPB, 128], BF, tag="eps_d", name="eps_d")
                    nc.gpsimd.dma_start(out=eps_d[:], in_=eps_sb[:])
                    nc.gpsimd.dma_start(
                        out=pqT1[115:116, :].rearrange("p (c q) -> p c q",
                                                       q=128),
                        in_=eps_d[:].rearrange("(r c) q -> r c q", r=1))
                    # e^M over this batch's tokens
                    eMp = st.tile([128, 1], FP, tag="eMp", name="eMp")
                    nc.vector.tensor_reduce(eMp[:], emx[:], axis=AX.X,
                                            op=ALU.max)
                    eMa = st.tile([128, 1], FP, tag="eMa", name="eMa")
                    nc.gpsimd.partition_all_reduce(
                        eMa[:], eMp[:], channels=128,
                        reduce_op=bass_isa.ReduceOp.max)
                    # ctx^T [82, 244]: (v|1)^T @ (pk|1)
                    ctxT = ctxp.tile([82, 244], FP, tag="ctx", name="ctx")
                    for cc in range(CPB):
                        c = b * CPB + cc
                        nc.tensor.matmul(
                            ctxT[:],
                            Vb[:, c * XS + h * HS: c * XS + h * HS + 82],
                            pk[:, cc * KS: cc * KS + 244],
                            start=(cc == 0), stop=(cc == CPB - 1))
                    epsv = st.tile([82, 1], FP, tag="epsv", name="epsv")
                    nc.vector.tensor_scalar(out=epsv[:], in0=ctxT[:, 243:244],
                                            scalar1=eMa[0:82, :], scalar2=EPS,
                                            op0=ALU.mult, op1=ALU.mult)
                    # ctx_sb [82, 244]: cols 0:243 ctx + eps, col 243 = G
                    ctx_sb = cxs.tile([82, 244], BF, tag="ctx_sb",
                                      name="ctx_sb")
                    nc.vector.tensor_scalar(out=ctx_sb[:, 0:NF],
                                            in0=ctxT[:, 0:NF],
                                            scalar1=epsv[:], scalar2=None,
                                            op0=ALU.add)
                    gcol = st.tile([82, 1], FP, tag="gcol", name="gcol")
                    nc.vector.tensor_reduce(gcol[:], ctx_sb[:, 0:NF],
                                            axis=AX.X, op=ALU.add)
                    nc.any.tensor_copy(ctx_sb[:, 243:244], gcol[:])
                    # j-major ctx blocks; ctxj1 row 115 = G
                    t0 = tpp.tile([128, 82], BF, tag="tp", name="tp")
                    nc.tensor.transpose(t0[0:128, 0:82], ctx_sb[:, 0:128],
                                        ident[0:82, 0:82])
                    ctxj0 = cxs.tile([128, 82], BF, tag="ctxj0", name="ctxj0")
                    nc.any.tensor_copy(ctxj0[:], t0[0:128, 0:82])
                    t1 = tpp.tile([116, 82], BF, tag="tp", name="tp")
                    nc.tensor.transpose(t1[0:116, 0:82], ctx_sb[:, 128:244],
                                        ident[0:82, 0:82])
                    ctxj1 = cxs.tile([116, 82], BF, tag="ctxj1", name="ctxj1")
                    nc.any.tensor_copy(ctxj1[:], t1[0:116, 0:82])
                    # A = pq @ ctx + epsn x G ; out = A[:, :81]/A[:, 81] + res
                    # two chunks share one PSUM tile -> one strided recip
                    for cp in range(CPB // 2):
                        A = app.tile([128, 164], FP, tag="A", name="A")
                        for k in range(2):
                            cc = 2 * cp + k
                            csl = slice(cc * 128, (cc + 1) * 128)
                            asl = slice(k * 82, k * 82 + 82)
                            nc.tensor.matmul(A[:, asl], pqT0[:, csl],
                                             ctxj0[:], start=True, stop=False)
                            nc.tensor.matmul(A[:, asl], pqT1[:, csl],
                                             ctxj1[:], start=False, stop=True)
                        dinv = st.tile([128, 2], FP, tag="dinv", name="dinv")
                        nc.vector.reciprocal(
                            dinv[:],
                            A[:].rearrange("p (k e) -> p k e", e=82)[:, :, 81])
                        for k in range(2):
                            c = b * CPB + 2 * cp + k
                            ysl = Y[:, c * YS + h * F: c * YS + (h + 1) * F]
                            xsl = Xres[:, c * YS + h * F: c * YS + (h + 1) * F]
                            nc.vector.scalar_tensor_tensor(
                                out=ysl, in0=A[:, k * 82: k * 82 + 81],
                                scalar=dinv[:, k:k + 1], in1=xsl,
                                op0=ALU.mult, op1=ALU.add)

    def layer_norm(self, Y, bf16_out, diag_pool=None):
        """In-place LN on Y.  Returns (padded bf16 tile or None, nbh or None)."""
        nc, st = self.nc, self.P["st"]
        need_diag = diag_pool is not None
        per_head = need_diag and self.ln_trivial
        sw = st.tile([128, 18 * NCH], FP, tag="sw", name="sw")
        mvg = st.tile([128, 2 * NCH], FP, tag="mvg", name="mvg")
        for c in range(NCH):
            if per_head:
                for h in range(H):
                    nc.vector.bn_stats(
                        out=sw[:, c * 18 + 6 * h: c * 18 + 6 * h + 6],
                        in_=Y[:, c * YS + h * F: c * YS + (h + 1) * F])
                nc.vector.bn_aggr(out=mvg[:, 2 * c:2 * c + 2],
                                  in_=sw[:, c * 18:(c + 1) * 18])
            else:
                nc.vector.bn_stats(out=sw[:, c * 18: c * 18 + 6],
                                   in_=Y[:, c * YS:(c + 1) * YS])
                nc.vector.bn_aggr(out=mvg[:, 2 * c:2 * c + 2],
                                  in_=sw[:, c * 18: c * 18 + 6])
        mvv = mvg[:].rearrange("p (c t) -> p c t", t=2)
        mu, var = mvv[:, :, 0], mvv[:, :, 1]
        sd = st.tile([128, NCH], FP, tag="lnsd", name="lnsd")
        nc.scalar.activation(sd[:], var, ACTF.Sqrt, bias=self.C["lneps"][:])
        rs = st.tile([128, NCH], FP, tag="lnrs", name="lnrs")
        nc.vector.reciprocal(rs[:], sd[:])
        nmr = st.tile([128, NCH], FP, tag="lnnmr", name="lnnmr")
        nc.vector.scalar_tensor_tensor(out=nmr[:], in0=mu, scalar=-1.0,
                                       in1=rs[:], op0=ALU.mult, op1=ALU.mult)
        nbh = None
        if need_diag and self.ln_trivial:
            rs2 = st.tile([128, NCH], FP, tag="lnrs2", name="lnrs2")
            nc.vector.tensor_mul(rs2[:], rs[:], rs[:])
            nbh = self.diag_stats(None, diag_pool, raw=False, sw=sw, mvg=mvg,
                                  rs2=rs2)
        Xb = None
        if bf16_out:
            Xb = self.P["xbf"].tile([128, NCH * XS], BF, tag="xbf", name="xbf")
            v3 = Xb[:].rearrange("p (c x) -> p c x", x=HS)
            nc.vector.memset(v3[:, :, F], 1.0)
        for c in range(NCH):
            sl = Y[:, c * YS:(c + 1) * YS]
            nc.any.tensor_scalar(out=sl, in0=sl, scalar1=rs[:, c:c + 1],
                                 scalar2=nmr[:, c:c + 1],
                                 op0=ALU.mult, op1=ALU.add)
            if not self.ln_trivial:
                nc.vector.tensor_mul(sl, sl, self.C["lnw"][:])
                nc.vector.tensor_add(sl, sl, self.C["lnb"][:])
        if bf16_out:
            for g in range(NCH // 4):
                nc.any.tensor_copy(self._pad_out_view(Xb, g),
                                   self._flat_in_view(Y, g))
        if need_diag and not self.ln_trivial:
            nbh = [diag_pool.tile([128, NCH], FP, tag=f"nbh{h}", name=f"nbh{h}")
                   for h in range(H)]
            for h in range(H):
                for c in range(NCH):
                    sl = Y[:, c * YS + h * F: c * YS + (h + 1) * F]
                    s = st.tile([128, F], FP, tag="sqh", name="sqh")
                    nc.vector.tensor_mul(s[:], sl, sl)
                    nc.vector.tensor_reduce(nbh[h][:, c:c + 1], s[:],
                                            axis=AX.X, op=ALU.add)
                nc.vector.tensor_scalar_mul(nbh[h][:], nbh[h][:], -DSCALE)
        return Xb, nbh

    def ff(self, Yb, FFIN, Ynew, w1h, w2k, b1c):
        """Ynew = gelu(FFIN@w1+b1)@w2 + b2 + FFIN."""
        nc = self.nc
        with ExitStack() as ps_ctx:
            f1p = ps_ctx.enter_context(
                self.tc.tile_pool(name="f1p", bufs=3, space="PSUM"))
            f2p = ps_ctx.enter_context(
                self.tc.tile_pool(name="f2p", bufs=2, space="PSUM"))
            # per-head feature-major DMA transposes (tags shared with the
            # attention head-transposes; never live at the same time)
            fth = self.transpose_heads(Yb)
            for ng in range(NT // 512):
                gts = []
                for kk in range(8):
                    mw = 128 if kk < 7 else 76
                    f1 = f1p.tile([128, 512], FP, tag="f1", name="f1")
                    for h in range(H):
                        nc.tensor.matmul(f1[0:mw, :],
                                         w1h[h][:, kk * 128: kk * 128 + mw],
                                         fth[h][0:F, ng * 512:(ng + 1) * 512],
                                         start=(h == 0), stop=(h == H - 1))
                    gt = self.P["gel"].tile([128, 512], BF, tag="g", name="g")
                    if kk == 7:
                        nc.vector.memset(gt[64:128, :], 0.0)
                    nc.scalar.activation(gt[0:mw, :], f1[0:mw, :], ACTF.Gelu,
                                         bias=b1c[0:mw, kk:kk + 1])
                    if kk == 7:
                        nc.vector.memset(gt[96:97, :], 1.0)
                    gts.append(gt)
                for j in range(4):
                    c = ng * 4 + j
                    f2 = f2p.tile([128, D], FP, tag="f2", name="f2")
                    for kk in range(8):
                        kw = 128 if kk < 7 else 97
                        nc.tensor.matmul(f2[:],
                                         gts[kk][0:kw, j * 128:(j + 1) * 128],
                                         w2k[kk][:],
                                         start=(kk == 0), stop=(kk == 7))
                    nc.vector.tensor_add(Ynew[:, c * YS:(c + 1) * YS], f2[:],
                                         FFIN[:, c * YS:(c + 1) * YS])


# ---------------- host side ----------------
def _prep_inputs(patches, ln_w, ln_b, enc_proj, enc_w1, enc_b1, enc_w2, enc_b2,
                 dec1_proj, dec2_proj, dec_w1, dec_b1, dec_w2, dec_b2):
    bf = ml_dtypes.bfloat16
    p = np.ascontiguousarray(patches).reshape(L, 2, B, D)
    projs = [enc_proj[0], enc_proj[1], dec1_proj[0], dec2_proj[0],
             dec1_proj[1], dec2_proj[1]]
    projt = np.stack([(np.asarray(pr).T * DN) for pr in projs]).astype(bf)
    w1s = np.stack([enc_w1[0], enc_w1[1], dec_w1[0], dec_w1[1]]).astype(bf)
    w2e = np.zeros((4, 993, D), np.float32)
    b1c = np.zeros((4, 128, 8), np.float32)
    for i, (w2, b1, b2) in enumerate([
            (enc_w2[0], enc_b1[0], enc_b2[0]), (enc_w2[1], enc_b1[1], enc_b2[1]),
            (dec_w2[0], dec_b1[0], dec_b2[0]), (dec_w2[1], dec_b1[1], dec_b2[1])]):
        w2e[i, :FFH] = np.asarray(w2)
        w2e[i, 992] = np.asarray(b2)
        b1p = np.zeros(1024, np.float32)
        b1p[:FFH] = np.asarray(b1)
        b1c[i] = b1p.reshape(8, 128).T
    w2e = w2e.astype(bf)
    lnw = np.tile(np.asarray(ln_w, np.float32)[None, :], (128, 1))
    lnb = np.tile(np.asarray(ln_b, np.float32)[None, :], (128, 1))
    ln_trivial = bool(np.all(np.asarray(ln_w) == 1.0)
                      and np.all(np.asarray(ln_b) == 0.0))
    in_maps = []
    for c in range(NCORES):
        bs = slice(NB * c, NB * (c + 1))
        xin = np.ascontiguousarray(
            p[:, 0, bs].transpose(1, 0, 2).reshape(NT, D)).astype(np.float32)
        xout = np.ascontiguousarray(
            p[:, 1, bs].transpose(1, 0, 2).reshape(NT, D)).astype(np.float32)
        in_maps.append(dict(xin=xin, xout=xout, projt=projt, w1=w1s, w2e=w2e,
                            b1c=b1c, lnw=lnw, lnb=lnb))
    return in_maps, ln_trivial


def kernel(**inputs):
    in_maps, ln_trivial = _prep_inputs(**inputs)
    key = ("nc", ln_trivial)
    if key not in _cache:
        _cache[key] = _build(ln_trivial)
    nc = _cache[key]
    res = run_bass_kernel_spmd(
        nc, in_maps, list(range(NCORES)),
        trace=bool(int(os.environ.get("KERNEL_TRACE", "0"))))
    kernel._last_result = res
    out = np.concatenate([res.results[c]["out"] for c in range(NCORES)], axis=0)
    return out


# revision 39
# speedup vs baseline: 1.4976x; 1.4976x over previous
"""Trainium2 Bass kernel for nn_AttnBYOL (Performer linear-attention BYOL net).

Self-contained: takes FULL inputs, shards batch B=32 across 8 NeuronCores
(4 batches/core), runs one SPMD Bass/Tile program, gathers full output.

v3 design notes:
- No max subtraction anywhere: pq = exp(x@proj) raw (range-checked on real
  data), pk = exp(x@proj - diag).  The reference's q-side row-max and +eps
  are reproduced exactly through a rank-1 correction A += epsn x G with
  epsn[n] = eps*maxrow(pk)[n]*e^{2*diag[n]}; the k-side +eps enters as a
  per-partition scalar add (eps*e^M*Vsum) folded into the ctx PSUM->SBUF
  copy.  The rank-1 rides as row 115 of pq^T (epsn, via a DRAM-bounced
  DMA) against row 115 of the j-major ctx block (G, via an extra ctx_sb
  column through the transpose), so pass 3 is two matmuls per chunk.
- Activations keep a padded bf16 layout with 128-wide head slots so every
  head transpose is a single [128,128] DMA-transpose (XBAR) instead of a
  PE matmul + PSUM copy; ones columns at slot offset 81 give v_aug/ksum
  for free.
- ctx accumulates e-major ([82, 244]) with v_aug as the 82-wide stationary;
  pq^T comes from a j-major projection matmul (projection stationary).
- LayerNorm: one 3-group bn_stats per chunk + per-chunk bn_aggr; the
  attention diag comes from closed-form even/odd group algebra (no
  per-head bn_aggr).  bf16 copies are 4-chunk batched 4D-AP copies.
"""
import os
import numpy as np
import ml_dtypes
from contextlib import ExitStack

import concourse.bass as bass
import concourse.tile as tile
from concourse import bacc, mybir, masks, bass_isa
from concourse.bass_utils import run_bass_kernel_spmd

FP = mybir.dt.float32
BF = mybir.dt.bfloat16
AX = mybir.AxisListType
ALU = mybir.AluOpType
ACTF = mybir.ActivationFunctionType

B, L, H, F, D = 32, 1024, 3, 81, 243
NF, FFH = 243, 972
NE, ND = 2, 2
NCORES = 8
NB = B // NCORES          # batches per core
NT = NB * L               # tokens per core (4096)
NCH = NT // 128           # 32 token chunks
CPB = L // 128            # 8 chunks per batch
DN = float(F) ** -0.25    # 1/3
DSCALE = 0.5 * DN * DN    # 1/18
EPS = 1e-4
LOG_EPS = float(np.log(EPS))
LNEPS = 1e-5
XS = 384                  # bf16 padded chunk stride (3 x 128 head slots)
HS = 128                  # head slot stride (81 data + ones col + pad)
KS = 256                  # pk chunk stride (243 + ones col + pad)
YS = 243                  # fp32 activation chunk stride
NEC = (F + 1) // 2        # bn_stats even-group count (41)
NOC = F // 2              # bn_stats odd-group count (40)

_cache = {}


def _build(ln_trivial: bool):
    nc = bacc.Bacc("TRN2", target_bir_lowering=False, debug=False,
                   enable_asserts=False, num_devices=NCORES)

    d_xin = nc.dram_tensor("xin", [NT, D], FP, kind="ExternalInput").ap()
    d_xout = nc.dram_tensor("xout", [NT, D], FP, kind="ExternalInput").ap()
    d_projt = nc.dram_tensor("projt", [6, 84, NF], BF, kind="ExternalInput").ap()
    d_w1 = nc.dram_tensor("w1", [4, D, FFH], BF, kind="ExternalInput").ap()
    d_w2e = nc.dram_tensor("w2e", [4, 993, D], BF, kind="ExternalInput").ap()
    d_b1c = nc.dram_tensor("b1c", [4, 128, 8], FP, kind="ExternalInput").ap()
    d_lnw = nc.dram_tensor("lnw", [128, D], FP, kind="ExternalInput").ap()
    d_lnb = nc.dram_tensor("lnb", [128, D], FP, kind="ExternalInput").ap()
    d_out = nc.dram_tensor("out", [NT, D], FP, kind="ExternalOutput").ap()

    with TileKernel(nc, ln_trivial) as k:
        k.run(d_xin, d_xout, d_projt, d_w1, d_w2e, d_b1c, d_lnw, d_lnb, d_out)

    nc.compile()
    return nc


class TileKernel:
    def __init__(self, nc, ln_trivial):
        self.nc = nc
        self.ln_trivial = ln_trivial
        self.ctx = ExitStack()

    def __enter__(self):
        self.tc = self.ctx.enter_context(tile.TileContext(self.nc))
        return self

    def __exit__(self, *a):
        return self.ctx.__exit__(*a)

    def pool(self, name, bufs, space="SBUF"):
        return self.ctx.enter_context(
            self.tc.tile_pool(name=name, bufs=bufs, space=space))

    # ================= top level =================
    def run(self, d_xin, d_xout, d_projt, d_w1, d_w2e, d_b1c, d_lnw, d_lnb,
            d_out):
        nc = self.nc

        const = self.pool("const", 1)
        wpool = self.pool("wts", 1)
        resid = self.pool("resid", 2)           # fp32 [128, 32*243] streams
        xbf = self.pool("xbf", 1)               # padded bf16 [128, 32*384]
        xt = self.pool("xt", 1)                 # transposed activations
        lth_p = self.pool("lth", 1)             # persistent l_in^T
        pqp = self.pool("pq", 3)                # pq^T j-major (per b,h)
        pkp = self.pool("pk", 2)                # pk token-major padded
        gel = self.pool("gelu", 10)
        st = self.pool("st", 4)                 # small stats tiles
        nbp = self.pool("nb", 2)                # -diag per head
        nbl = self.pool("nbL", 1)               # -diag of l_in final
        cxs = self.pool("cxs", 3)               # ctx_sb/ctxj/eps_sb

        ident = const.tile([128, 128], BF)
        masks.make_identity(nc, ident[:])
        zero_c = const.tile([128, 1], FP)
        nc.vector.memset(zero_c[:], 0.0)
        lneps_c = const.tile([128, 1], FP)
        nc.vector.memset(lneps_c[:], LNEPS)
        logeps_c = const.tile([128, 1], FP)
        nc.vector.memset(logeps_c[:], LOG_EPS)
        self.C = dict(ident=ident, zero=zero_c, lneps=lneps_c, logeps=logeps_c)
        if not self.ln_trivial:
            lnw_t = const.tile([128, D], FP)
            lnb_t = const.tile([128, D], FP)
            nc.sync.dma_start(out=lnw_t[:], in_=d_lnw)
            nc.sync.dma_start(out=lnb_t[:], in_=d_lnb)
            self.C["lnw"] = lnw_t
            self.C["lnb"] = lnb_t
        projt_t = []
        for a in range(6):
            # rows 0:81 = proj^T * dn, row 81 = 0 (skips the ones column of
            # the padded x^T), rows 82:84 = 1 (pick up the compensated -diag
            # rows so pk's exp needs no bias)
            t = const.tile([84, NF], BF, tag=f"projt{a}", name=f"projt{a}")
            nc.sync.dma_start(out=t[:], in_=d_projt[a])
            projt_t.append(t)

        self.P = dict(resid=resid, xbf=xbf, xt=xt, lth=lth_p, pq=pqp, pk=pkp,
                      gel=gel, st=st, nb=nbp, nbl=nbl, cxs=cxs, w=wpool)

        def load_ff_w(i):
            w1h = []
            for h in range(H):
                t = wpool.tile([F, FFH], BF, tag=f"w1h{h}", name=f"w1h{h}")
                nc.sync.dma_start(out=t[:], in_=d_w1[i, h * F:(h + 1) * F])
                w1h.append(t)
            w2k = []
            for kk in range(8):
                kw = 128 if kk < 7 else 97
                t = wpool.tile([kw, D], BF, tag=f"w2k{kk}", name=f"w2k{kk}")
                nc.sync.dma_start(out=t[:], in_=d_w2e[i, kk * 128: kk * 128 + kw])
                w2k.append(t)
            b1c = wpool.tile([128, 8], FP, tag="b1c", name="b1c")
            nc.sync.dma_start(out=b1c[:], in_=d_b1c[i])
            return w1h, w2k, b1c

        # ---------------- encoder ----------------
        X = resid.tile([128, NCH * YS], FP, tag="resid", name="resid")
        nc.sync.dma_start(out=X[:].rearrange("p (c d) -> p c d", d=YS),
                          in_=d_xin.rearrange("(c p) d -> p c d", p=128))
        Xb = self.make_bf16(X)
        nbh = self.diag_stats(X, self.P["nb"], raw=True)
        self.write_diag_cols(Xb, nbh)

        for i in range(NE):
            xth = self.transpose_heads(Xb)
            Y = resid.tile([128, NCH * YS], FP, tag="resid", name="resid")
            self.attention(projt_t[i], xth, nbh, Xb, X, Y)
            Yb = self.layer_norm(Y, bf16_out=True)[0]
            w1h, w2k, b1c = load_ff_w(i)
            X2 = resid.tile([128, NCH * YS], FP, tag="resid", name="resid")
            self.ff(Yb, Y, X2, w1h, w2k, b1c)
            last = (i == NE - 1)
            Xb, nbh = self.layer_norm(X2, bf16_out=True,
                                      diag_pool=(self.P["nbl"] if last
                                                 else self.P["nb"]))
            self.write_diag_cols(Xb, nbh)
            X = X2

        # persistent transposed l_in for the decoder cross attentions
        lth = [self.P["lth"].tile([84, NT], BF, tag=f"lth{h}", name=f"lth{h}")
               for h in range(H)]
        self.transpose_heads(Xb, dst=lth)
        nbhL = nbh

        # ---------------- decoder ----------------
        X = resid.tile([128, NCH * YS], FP, tag="resid", name="resid")
        nc.sync.dma_start(out=X[:].rearrange("p (c d) -> p c d", d=YS),
                          in_=d_xout.rearrange("(c p) d -> p c d", p=128))
        Xb = self.make_bf16(X)
        nbh = self.diag_stats(X, self.P["nb"], raw=True)
        self.write_diag_cols(Xb, nbh)

        for i in range(ND):
            xth = self.transpose_heads(Xb)
            Y = resid.tile([128, NCH * YS], FP, tag="resid", name="resid")
            self.attention(projt_t[2 + 2 * i], xth, nbh, Xb, X, Y)
            A1b = self.layer_norm(Y, bf16_out=True)[0]          # a1
            Y2 = resid.tile([128, NCH * YS], FP, tag="resid", name="resid")
            self.attention(projt_t[3 + 2 * i], lth, nbhL, A1b, Y, Y2)
            A2b = self.layer_norm(Y2, bf16_out=True)[0]         # a2
            w1h, w2k, b1c = load_ff_w(2 + i)
            X2 = resid.tile([128, NCH * YS], FP, tag="resid", name="resid")
            self.ff(A2b, Y2, X2, w1h, w2k, b1c)
            last = (i == ND - 1)
            if last:
                self.layer_norm(X2, bf16_out=False)
            else:
                Xb, nbh = self.layer_norm(X2, bf16_out=True,
                                          diag_pool=self.P["nb"])
                self.write_diag_cols(Xb, nbh)
            X = X2

        nc.sync.dma_start(out=d_out.rearrange("(c p) d -> p c d", p=128),
                          in_=X[:].rearrange("p (c d) -> p c d", d=YS))

    # ================= building blocks =================
    def _pad_out_view(self, Xb, g):
        """4-chunk padded output view: [128, 4, 3, 81] at chunk group g."""
        return Xb[:, g * 4 * XS:(g + 1) * 4 * XS].rearrange(
            "p (c h y) -> p c h y", h=H, y=HS)[:, :, :, 0:F]

    def _flat_in_view(self, X, g):
        """4-chunk fp32 input view: [128, 4, 3, 81] at chunk group g."""
        return X[:, g * 4 * YS:(g + 1) * 4 * YS].rearrange(
            "p (c h f) -> p c h f", h=H, f=F)

    def make_bf16(self, X, Xb=None):
        """fp32 token-major -> padded bf16 (128-stride head slots, ones)."""
        nc = self.nc
        if Xb is None:
            Xb = self.P["xbf"].tile([128, NCH * XS], BF, tag="xbf", name="xbf")
        v3 = Xb[:].rearrange("p (c x) -> p c x", x=HS)
        nc.vector.memset(v3[:, :, F], 1.0)
        for g in range(NCH // 4):
            nc.gpsimd.tensor_copy(self._pad_out_view(Xb, g),
                                  self._flat_in_view(X, g))
        return Xb

    def write_diag_cols(self, Xb, nbh):
        """Write compensated bf16(-diag) into head-slot cols 82/83 so the
        zk matmul contraction applies the diag bias exactly."""
        nc, st = self.nc, self.P["st"]
        v4 = Xb[:].rearrange("p (c h x) -> p c h x", h=H, x=HS)
        for h in range(H):
            co = st.tile([128, NCH], BF, tag="dco", name="dco")
            nc.vector.tensor_copy(co[:], nbh[h][:])
            re = st.tile([128, NCH], FP, tag="dre", name="dre")
            nc.vector.tensor_sub(re[:], nbh[h][:], co[:])
            nc.vector.tensor_copy(v4[:, :, h, 82], co[:])
            nc.vector.tensor_copy(v4[:, :, h, 83], re[:])

    def diag_stats(self, X, pool, raw, sw=None, mvg=None, rs2=None):
        """nbh[h] = -DSCALE * ||x_h||^2 (raw) or of the LN output (from
        even/odd bn_stats group algebra).  X used only when sw is None."""
        nc, st = self.nc, self.P["st"]
        if sw is None:
            sw = st.tile([128, 18 * NCH], FP, tag="sw", name="sw")
            for c in range(NCH):
                for h in range(H):
                    nc.vector.bn_stats(
                        out=sw[:, c * 18 + 6 * h: c * 18 + 6 * h + 6],
                        in_=X[:, c * YS + h * F: c * YS + (h + 1) * F])
        swv = sw[:].rearrange("p (c s) -> p c s", s=18)
        mu81 = None
        if not raw:
            mvv = mvg[:].rearrange("p (c t) -> p c t", t=2)
            mu = mvv[:, :, 0]
            mu81 = st.tile([128, NCH], FP, tag="mu81", name="mu81")
            nc.vector.tensor_scalar_mul(mu81[:], mu, float(D))
        nbh = [pool.tile([128, NCH], FP, tag=f"nbh{h}", name=f"nbh{h}")
               for h in range(H)]
        for h in range(H):
            me, m2e = swv[:, :, 6 * h + 1], swv[:, :, 6 * h + 2]
            mo, m2o = swv[:, :, 6 * h + 4], swv[:, :, 6 * h + 5]
            a = st.tile([128, NCH], FP, tag="dga", name="dga")
            q = st.tile([128, NCH], FP, tag="dgq", name="dgq")
            nc.vector.tensor_mul(a[:], me, me)
            nc.vector.scalar_tensor_tensor(out=q[:], in0=a[:], scalar=float(NEC),
                                           in1=m2e, op0=ALU.mult, op1=ALU.add)
            nc.vector.tensor_mul(a[:], mo, mo)
            nc.vector.scalar_tensor_tensor(out=a[:], in0=a[:], scalar=float(NOC),
                                           in1=m2o, op0=ALU.mult, op1=ALU.add)
            nc.vector.tensor_add(q[:], q[:], a[:])      # sum x^2 over head
            if raw:
                nc.vector.tensor_scalar_mul(nbh[h][:], q[:], -DSCALE)
                continue
            # s = 41*me + 40*mo ; u = (q + mu*(D*mu - 2*s)) * rs^2
            s = st.tile([128, NCH], FP, tag="dgs", name="dgs")
            nc.vector.tensor_scalar_mul(s[:], mo, float(NOC))
            nc.vector.scalar_tensor_tensor(out=s[:], in0=me, scalar=float(NEC),
                                           in1=s[:], op0=ALU.mult, op1=ALU.add)
            nc.vector.scalar_tensor_tensor(out=s[:], in0=s[:], scalar=-2.0,
                                           in1=mu81[:], op0=ALU.mult,
                                           op1=ALU.add)
            mvv = mvg[:].rearrange("p (c t) -> p c t", t=2)
            nc.vector.tensor_mul(s[:], s[:], mvv[:, :, 0])
            nc.vector.tensor_add(s[:], s[:], q[:])
            nc.vector.tensor_mul(s[:], s[:], rs2[:])
            nc.vector.tensor_scalar_mul(nbh[h][:], s[:], -DSCALE)
        return nbh

    def transpose_heads(self, Xb, dst=None):
        """padded token-major -> per-head feature-major via PE transposes.
        Transposes 84 slot columns: rows 0:81 features, row 81 the ones
        column (unused), rows 82:84 the compensated -diag rows."""
        nc = self.nc
        ident = self.C["ident"]
        if dst is None:
            dst = [self.P["xt"].tile([84, NT], BF, tag=f"xt{h}",
                                     name=f"xth{h}")
                   for h in range(H)]
        with ExitStack() as mctx:
            tp = mctx.enter_context(
                self.tc.tile_pool(name="tph", bufs=3, space="PSUM"))
            for c in range(NCH):
                for h in range(H):
                    ps = tp.tile([84, 128], BF, tag="tph", name="tph")
                    nc.tensor.transpose(
                        ps[0:84, 0:128],
                        Xb[:, c * XS + h * HS: c * XS + h * HS + 84],
                        ident[:, :])
                    nc.any.tensor_copy(dst[h][:, c * 128:(c + 1) * 128],
                                       ps[0:84, 0:128])
        return dst

    def attention(self, projt, xth, nbh, Vb, Xres, Y):
        """Y = performer_attention(q=k=xth-owner, v=Vb) + Xres.

        Batch-outer loop: each batch finishes all 3 heads before the next
        starts, so the consumer LayerNorm's per-chunk stats can overlap the
        remaining batches (keeps PE fed and the HAM warm)."""
        nc = self.nc
        ident = self.C["ident"]
        st, cxs = self.P["st"], self.P["cxs"]
        with ExitStack() as ps_ctx:
            zdp = ps_ctx.enter_context(
                self.tc.tile_pool(name="zdp", bufs=3, space="PSUM"))
            tpp = ps_ctx.enter_context(
                self.tc.tile_pool(name="tpp", bufs=1, space="PSUM"))
            ctxp = ps_ctx.enter_context(
                self.tc.tile_pool(name="ctxp", bufs=2, space="PSUM"))
            app = ps_ctx.enter_context(
                self.tc.tile_pool(name="app", bufs=2, space="PSUM"))
            drp = ps_ctx.enter_context(
                self.tc.tile_pool(name="drp", bufs=2, space="DRAM"))
            for b in range(NB):
                tb = b * L           # first token of the batch
                for h in range(H):
                    # ---- pq^T = exp(projt^T @ x^T), j-major, no bias ----
                    # pqT1 row 115 carries epsn (q-side eps rank-1 factor)
                    pqT0 = self.P["pq"].tile([128, L], BF, tag="pqT0",
                                             name="pqT0")
                    pqT1 = self.P["pq"].tile([116, L], BF, tag="pqT1",
                                             name="pqT1")
                    for jb, jw, pq in ((0, 128, pqT0), (1, 115, pqT1)):
                        for t in range(L // 512):
                            zt = zdp.tile([128, 512], FP, tag="zd", name="zd")
                            nc.tensor.matmul(
                                zt[0:jw, :],
                                projt[:, jb * 128: jb * 128 + jw],
                                xth[h][:, tb + t * 512: tb + (t + 1) * 512],
                                start=True, stop=True)
                            nc.scalar.activation(
                                pq[0:jw, t * 512:(t + 1) * 512],
                                zt[0:jw, :], ACTF.Exp,
                                bias=self.C["zero"][0:jw, :])
                    # ---- pk = exp(zd - diag), token-major; the -diag enters
                    # through the compensated rows 82:84 of xth/projt so the
                    # exp is bias-free and processes two chunks per op ----
                    pk = self.P["pk"].tile([128, CPB * KS], BF, tag="pk",
                                           name="pk")
                    nc.vector.memset(
                        pk[:].rearrange("p (c x) -> p c x", x=KS)[:, :, NF],
                        1.0)
                    for cp in range(CPB // 2):
                        zk2 = zdp.tile([128, 2 * NF], FP, tag="zd", name="zd")
                        for k in range(2):
                            c = b * CPB + 2 * cp + k
                            nc.tensor.matmul(
                                zk2[:, k * NF:(k + 1) * NF],
                                xth[h][:, c * 128:(c + 1) * 128],
                                projt[:, 0:NF], start=True, stop=True)
                        nc.scalar.activation(
                            pk[:, 2 * cp * KS: 2 * (cp + 1) * KS].rearrange(
                                "p (k x) -> p k x", x=KS)[:, :, 0:NF],
                            zk2[:].rearrange("p (k f) -> p k f", f=NF),
                            ACTF.Exp, bias=self.C["zero"][:])
                    nbs = nbh[h][:, b * CPB:(b + 1) * CPB]
                    mx = st.tile([128, CPB], FP, tag="mx", name="mx")
                    nc.vector.tensor_reduce(
                        mx[:],
                        pk[:].rearrange("p (c x) -> p c x", x=KS)[:, :, 0:NF],
                        axis=AX.X, op=ALU.max)
                    # ---- emx = mx*e^{diag} = e^{rm}; epsn = eps*emx (pq
                    # carries e^{-diag} too, so the eps factor simplifies) ----
                    ed = st.tile([128, CPB], FP, tag="ed", name="ed")
                    nc.scalar.activation(ed[:], nbs, ACTF.Exp, scale=-1.0,
                                         bias=self.C["zero"][:])
                    emx = st.tile([128, CPB], FP, tag="emx", name="emx")
                    nc.vector.tensor_mul(emx[:], mx[:], ed[:])
                    epsn = st.tile([128, CPB], BF, tag="epsn", name="epsn")
                    nc.vector.tensor_scalar_mul(epsn[:], emx[:], EPS)
                    # epsn -> pqT1 row 115 (transpose + DRAM bounce on the
                    # gpsimd SWDGE queue; matmul stationaries must start at
                    # partition 0/32/64 so a row tile is required)
                    tr = tpp.tile([CPB, 128], BF, tag="tp", name="tp")
                    nc.tensor.transpose(tr[0:CPB, 0:128], epsn[:, :],
                                        ident[:, :])
                    eps_sb = cxs.tile([CPB, 128], BF, tag="eps_sb",
                                      name="eps_sb")
                    nc.any.tensor_copy(eps_sb[:], tr[0:CPB, 0:128])
                    eps_d = drp.tile([CPB, 128], BF, tag="eps_d", name="eps_d")
                    nc.gpsimd.dma_start(out=eps_d[:], in_=eps_sb[:])
                    nc.gpsimd.dma_start(
                        out=pqT1[115:116, :].rearrange("p (c q) -> p c q",
                                                       q=128),
                        in_=eps_d[:].rearrange("(r c) q -> r c q", r=1))
                    # e^M over this batch's tokens
                    eMp = st.tile([128, 1], FP, tag="eMp", name="eMp")
                    nc.vector.tensor_reduce(eMp[:], emx[:], axis=AX.X,
                                            op=ALU.max)
                    eMa = st.tile([128, 1], FP, tag="eMa", name="eMa")
                    nc.gpsimd.partition_all_reduce(
                        eMa[:], eMp[:], channels=128,
                        reduce_op=bass_isa.ReduceOp.max)
                    # ctx^T [82, 244]: (v|1)^T @ (pk|1)
                    ctxT = ctxp.tile([82, 244], FP, tag="ctx", name="ctx")
                    for cc in range(CPB):
                        c = b * CPB + cc
                        nc.tensor.matmul(
                            ctxT[:],
                            Vb[:, c * XS + h * HS: c * XS + h * HS + 82],
                            pk[:, cc * KS: cc * KS + 244],
                            start=(cc == 0), stop=(cc == CPB - 1))
                    epsv = st.tile([82, 1], FP, tag="epsv", name="epsv")
                    nc.vector.tensor_scalar(out=epsv[:], in0=ctxT[:, 243:244],
                                            scalar1=eMa[0:82, :], scalar2=EPS,
                                            op0=ALU.mult, op1=ALU.mult)
                    # ctx_sb [82, 244]: cols 0:243 ctx + eps, col 243 = G
                    ctx_sb = cxs.tile([82, 244], BF, tag="ctx_sb",
                                      name="ctx_sb")
                    nc.vector.tensor_scalar(out=ctx_sb[:, 0:NF],
                                            in0=ctxT[:, 0:NF],
                                            scalar1=epsv[:], scalar2=None,
                                            op0=ALU.add)
                    gcol = st.tile([82, 1], FP, tag="gcol", name="gcol")
                    nc.vector.tensor_reduce(gcol[:], ctx_sb[:, 0:NF],
                                            axis=AX.X, op=ALU.add)
                    nc.any.tensor_copy(ctx_sb[:, 243:244], gcol[:])
                    # j-major ctx blocks; ctxj1 row 115 = G
                    t0 = tpp.tile([128, 82], BF, tag="tp", name="tp")
                    nc.tensor.transpose(t0[0:128, 0:82], ctx_sb[:, 0:128],
                                        ident[0:82, 0:82])
                    ctxj0 = cxs.tile([128, 82], BF, tag="ctxj0", name="ctxj0")
                    nc.any.tensor_copy(ctxj0[:], t0[0:128, 0:82])
                    t1 = tpp.tile([116, 82], BF, tag="tp", name="tp")
                    nc.tensor.transpose(t1[0:116, 0:82], ctx_sb[:, 128:244],
                                        ident[0:82, 0:82])
                    ctxj1 = cxs.tile([116, 82], BF, tag="ctxj1", name="ctxj1")
                    nc.any.tensor_copy(ctxj1[:], t1[0:116, 0:82])
                    # A = pq @ ctx + epsn x G ; out = A[:, :81]/A[:, 81] + res
                    # two chunks share one PSUM tile -> one strided recip
                    for cp in range(CPB // 2):
                        A = app.tile([128, 164], FP, tag="A", name="A")
                        for k in range(2):
                            cc = 2 * cp + k
                            csl = slice(cc * 128, (cc + 1) * 128)
                            asl = slice(k * 82, k * 82 + 82)
                            nc.tensor.matmul(A[:, asl], pqT0[:, csl],
                                             ctxj0[:], start=True, stop=False)
                            nc.tensor.matmul(A[:, asl], pqT1[:, csl],
                                             ctxj1[:], start=False, stop=True)
                        dinv = st.tile([128, 2], FP, tag="dinv", name="dinv")
                        nc.vector.reciprocal(
                            dinv[:],
                            A[:].rearrange("p (k e) -> p k e", e=82)[:, :, 81])
                        for k in range(2):
                            c = b * CPB + 2 * cp + k
                            ysl = Y[:, c * YS + h * F: c * YS + (h + 1) * F]
                            xsl = Xres[:, c * YS + h * F: c * YS + (h + 1) * F]
                            nc.vector.scalar_tensor_tensor(
                                out=ysl, in0=A[:, k * 82: k * 82 + 81],
                                scalar=dinv[:, k:k + 1], in1=xsl,
                                op0=ALU.mult, op1=ALU.add)

    def layer_norm(self, Y, bf16_out, diag_pool=None):
        """In-place LN on Y.  Returns (padded bf16 tile or None, nbh or None)."""
        nc, st = self.nc, self.P["st"]
        need_diag = diag_pool is not None
        per_head = need_diag and self.ln_trivial
        sw = st.tile([128, 18 * NCH], FP, tag="sw", name="sw")
        mvg = st.tile([128, 2 * NCH], FP, tag="mvg", name="mvg")
        for c in range(NCH):
            if per_head:
                for h in range(H):
                    nc.vector.bn_stats(
                        out=sw[:, c * 18 + 6 * h: c * 18 + 6 * h + 6],
                        in_=Y[:, c * YS + h * F: c * YS + (h + 1) * F])
                nc.vector.bn_aggr(out=mvg[:, 2 * c:2 * c + 2],
                                  in_=sw[:, c * 18:(c + 1) * 18])
            else:
                nc.vector.bn_stats(out=sw[:, c * 18: c * 18 + 6],
                                   in_=Y[:, c * YS:(c + 1) * YS])
                nc.vector.bn_aggr(out=mvg[:, 2 * c:2 * c + 2],
                                  in_=sw[:, c * 18: c * 18 + 6])
        mvv = mvg[:].rearrange("p (c t) -> p c t", t=2)
        mu, var = mvv[:, :, 0], mvv[:, :, 1]
        sd = st.tile([128, NCH], FP, tag="lnsd", name="lnsd")
        nc.scalar.activation(sd[:], var, ACTF.Sqrt, bias=self.C["lneps"][:])
        rs = st.tile([128, NCH], FP, tag="lnrs", name="lnrs")
        nc.vector.reciprocal(rs[:], sd[:])
        nmr = st.tile([128, NCH], FP, tag="lnnmr", name="lnnmr")
        nc.vector.scalar_tensor_tensor(out=nmr[:], in0=mu, scalar=-1.0,
                                       in1=rs[:], op0=ALU.mult, op1=ALU.mult)
        nbh = None
        if need_diag and self.ln_trivial:
            rs2 = st.tile([128, NCH], FP, tag="lnrs2", name="lnrs2")
            nc.vector.tensor_mul(rs2[:], rs[:], rs[:])
            nbh = self.diag_stats(None, diag_pool, raw=False, sw=sw, mvg=mvg,
                                  rs2=rs2)
        Xb = None
        if bf16_out:
            Xb = self.P["xbf"].tile([128, NCH * XS], BF, tag="xbf", name="xbf")
            v3 = Xb[:].rearrange("p (c x) -> p c x", x=HS)
            nc.vector.memset(v3[:, :, F], 1.0)
        for c in range(NCH):
            sl = Y[:, c * YS:(c + 1) * YS]
            nc.any.tensor_scalar(out=sl, in0=sl, scalar1=rs[:, c:c + 1],
                                 scalar2=nmr[:, c:c + 1],
                                 op0=ALU.mult, op1=ALU.add)
            if not self.ln_trivial:
                nc.vector.tensor_mul(sl, sl, self.C["lnw"][:])
                nc.vector.tensor_add(sl, sl, self.C["lnb"][:])
        if bf16_out:
            for g in range(NCH // 4):
                nc.gpsimd.tensor_copy(self._pad_out_view(Xb, g),
                                      self._flat_in_view(Y, g))
        if need_diag and not self.ln_trivial:
            nbh = [diag_pool.tile([128, NCH], FP, tag=f"nbh{h}", name=f"nbh{h}")
                   for h in range(H)]
            for h in range(H):
                for c in range(NCH):
                    sl = Y[:, c * YS + h * F: c * YS + (h + 1) * F]
                    s = st.tile([128, F], FP, tag="sqh", name="sqh")
                    nc.vector.tensor_mul(s[:], sl, sl)
                    nc.vector.tensor_reduce(nbh[h][:, c:c + 1], s[:],
                                            axis=AX.X, op=ALU.add)
                nc.vector.tensor_scalar_mul(nbh[h][:], nbh[h][:], -DSCALE)
        return Xb, nbh

    def ff(self, Yb, FFIN, Ynew, w1h, w2k, b1c):
        """Ynew = gelu(FFIN@w1+b1)@w2 + b2 + FFIN."""
        nc = self.nc
        with ExitStack() as ps_ctx:
            f1p = ps_ctx.enter_context(
                self.tc.tile_pool(name="f1p", bufs=3, space="PSUM"))
            f2p = ps_ctx.enter_context(
                self.tc.tile_pool(name="f2p", bufs=2, space="PSUM"))
            # per-head feature-major DMA transposes (tags shared with the
            # attention head-transposes; never live at the same time)
            fth = self.transpose_heads(Yb)
            for ng in range(NT // 512):
                gts = []
                for kk in range(8):
                    mw = 128 if kk < 7 else 76
                    f1 = f1p.tile([128, 512], FP, tag="f1", name="f1")
                    for h in range(H):
                        nc.tensor.matmul(f1[0:mw, :],
                                         w1h[h][:, kk * 128: kk * 128 + mw],
                                         fth[h][0:F, ng * 512:(ng + 1) * 512],
                                         start=(h == 0), stop=(h == H - 1))
                    gt = self.P["gel"].tile([128, 512], BF, tag="g", name="g")
                    if kk == 7:
                        nc.vector.memset(gt[64:128, :], 0.0)
                    nc.scalar.activation(gt[0:mw, :], f1[0:mw, :], ACTF.Gelu,
                                         bias=b1c[0:mw, kk:kk + 1])
                    if kk == 7:
                        nc.vector.memset(gt[96:97, :], 1.0)
                    gts.append(gt)
                for j in range(4):
                    c = ng * 4 + j
                    f2 = f2p.tile([128, D], FP, tag="f2", name="f2")
                    for kk in range(8):
                        kw = 128 if kk < 7 else 97
                        nc.tensor.matmul(f2[:],
                                         gts[kk][0:kw, j * 128:(j + 1) * 128],
                                         w2k[kk][:],
                                         start=(kk == 0), stop=(kk == 7))
                    nc.vector.tensor_add(Ynew[:, c * YS:(c + 1) * YS], f2[:],
                                         FFIN[:, c * YS:(c + 1) * YS])


# ---------------- host side ----------------
def _prep_inputs(patches, ln_w, ln_b, enc_proj, enc_w1, enc_b1, enc_w2, enc_b2,
                 dec1_proj, dec2_proj, dec_w1, dec_b1, dec_w2, dec_b2):
    bf = ml_dtypes.bfloat16
    p = np.ascontiguousarray(patches).reshape(L, 2, B, D)
    projs = [enc_proj[0], enc_proj[1], dec1_proj[0], dec2_proj[0],
             dec1_proj[1], dec2_proj[1]]
    projt = np.zeros((6, 84, NF), np.float32)
    for a, pr in enumerate(projs):
        projt[a, :F] = np.asarray(pr).T * DN
        projt[a, 82:84] = 1.0
    projt = projt.astype(bf)
    w1s = np.stack([enc_w1[0], enc_w1[1], dec_w1[0], dec_w1[1]]).astype(bf)
    w2e = np.zeros((4, 993, D), np.float32)
    b1c = np.zeros((4, 128, 8), np.float32)
    for i, (w2, b1, b2) in enumerate([
            (enc_w2[0], enc_b1[0], enc_b2[0]), (enc_w2[1], enc_b1[1], enc_b2[1]),
            (dec_w2[0], dec_b1[0], dec_b2[0]), (dec_w2[1], dec_b1[1], dec_b2[1])]):
        w2e[i, :FFH] = np.asarray(w2)
        w2e[i, 992] = np.asarray(b2)
        b1p = np.zeros(1024, np.float32)
        b1p[:FFH] = np.asarray(b1)
        b1c[i] = b1p.reshape(8, 128).T
    w2e = w2e.astype(bf)
    lnw = np.tile(np.asarray(ln_w, np.float32)[None, :], (128, 1))
    lnb = np.tile(np.asarray(ln_b, np.float32)[None, :], (128, 1))
    ln_trivial = bool(np.all(np.asarray(ln_w) == 1.0)
                      and np.all(np.asarray(ln_b) == 0.0))
    in_maps = []
    for c in range(NCORES):
        bs = slice(NB * c, NB * (c + 1))
        xin = np.ascontiguousarray(
            p[:, 0, bs].transpose(1, 0, 2).reshape(NT, D)).astype(np.float32)
        xout = np.ascontiguousarray(
            p[:, 1, bs].transpose(1, 0, 2).reshape(NT, D)).astype(np.float32)
        in_maps.append(dict(xin=xin, xout=xout, projt=projt, w1=w1s, w2e=w2e,
                            b1c=b1c, lnw=lnw, lnb=lnb))
    return in_maps, ln_trivial


def kernel(**inputs):
    in_maps, ln_trivial = _prep_inputs(**inputs)
    key = ("nc", ln_trivial)
    if key not in _cache:
        _cache[key] = _build(ln_trivial)
    nc = _cache[key]
    res = run_bass_kernel_spmd(
        nc, in_maps, list(range(NCORES)),
        trace=bool(int(os.environ.get("KERNEL_TRACE", "0"))))
    kernel._last_result = res
    out = np.concatenate([res.results[c]["out"] for c in range(NCORES)], axis=0)
    return out


# revision 40
# speedup vs baseline: 1.6555x; 1.1054x over previous
"""Trainium2 Bass kernel for nn_AttnBYOL (Performer linear-attention BYOL net).

Self-contained: takes FULL inputs, shards batch B=32 across 8 NeuronCores
(4 batches/core), runs one SPMD Bass/Tile program, gathers full output.

v3 design notes:
- No max subtraction anywhere: pq = exp(x@proj) raw (range-checked on real
  data), pk = exp(x@proj - diag).  The reference's q-side row-max and +eps
  are reproduced exactly through a rank-1 correction A += epsn x G with
  epsn[n] = eps*maxrow(pk)[n]*e^{2*diag[n]}; the k-side +eps enters as a
  per-partition scalar add (eps*e^M*Vsum) folded into the ctx PSUM->SBUF
  copy.  The rank-1 rides as row 115 of pq^T (epsn, via a DRAM-bounced
  DMA) against row 115 of the j-major ctx block (G, via an extra ctx_sb
  column through the transpose), so pass 3 is two matmuls per chunk.
- Activations keep a padded bf16 layout with 128-wide head slots so every
  head transpose is a single [128,128] DMA-transpose (XBAR) instead of a
  PE matmul + PSUM copy; ones columns at slot offset 81 give v_aug/ksum
  for free.
- ctx accumulates e-major ([82, 244]) with v_aug as the 82-wide stationary;
  pq^T comes from a j-major projection matmul (projection stationary).
- LayerNorm: one 3-group bn_stats per chunk + per-chunk bn_aggr; the
  attention diag comes from closed-form even/odd group algebra (no
  per-head bn_aggr).  bf16 copies are 4-chunk batched 4D-AP copies.
"""
import os
import numpy as np
import ml_dtypes
from contextlib import ExitStack

import concourse.bass as bass
import concourse.tile as tile
from concourse import bacc, mybir, masks, bass_isa
from concourse.bass_utils import run_bass_kernel_spmd

FP = mybir.dt.float32
BF = mybir.dt.bfloat16
AX = mybir.AxisListType
ALU = mybir.AluOpType
ACTF = mybir.ActivationFunctionType

B, L, H, F, D = 32, 1024, 3, 81, 243
NF, FFH = 243, 972
NE, ND = 2, 2
NCORES = 8
NB = B // NCORES          # batches per core
NT = NB * L               # tokens per core (4096)
NCH = NT // 128           # 32 token chunks
CPB = L // 128            # 8 chunks per batch
DN = float(F) ** -0.25    # 1/3
DSCALE = 0.5 * DN * DN    # 1/18
EPS = 1e-4
LOG_EPS = float(np.log(EPS))
LNEPS = 1e-5
XS = 384                  # bf16 padded chunk stride (3 x 128 head slots)
HS = 128                  # head slot stride (81 data + ones col + pad)
KS = 256                  # pk chunk stride (243 + ones col + pad)
YS = 243                  # fp32 activation chunk stride
NEC = (F + 1) // 2        # bn_stats even-group count (41)
NOC = F // 2              # bn_stats odd-group count (40)

_cache = {}


def _build(ln_trivial: bool):
    nc = bacc.Bacc("TRN2", target_bir_lowering=False, debug=False,
                   enable_asserts=False, num_devices=NCORES)

    d_xin = nc.dram_tensor("xin", [NT, D], FP, kind="ExternalInput").ap()
    d_xout = nc.dram_tensor("xout", [NT, D], FP, kind="ExternalInput").ap()
    d_projt = nc.dram_tensor("projt", [6, 84, NF], BF, kind="ExternalInput").ap()
    d_w1 = nc.dram_tensor("w1", [4, D, FFH], BF, kind="ExternalInput").ap()
    d_w2e = nc.dram_tensor("w2e", [4, 993, D], BF, kind="ExternalInput").ap()
    d_b1c = nc.dram_tensor("b1c", [4, 128, 8], FP, kind="ExternalInput").ap()
    d_lnw = nc.dram_tensor("lnw", [128, D], FP, kind="ExternalInput").ap()
    d_lnb = nc.dram_tensor("lnb", [128, D], FP, kind="ExternalInput").ap()
    d_out = nc.dram_tensor("out", [NT, D], FP, kind="ExternalOutput").ap()

    with TileKernel(nc, ln_trivial) as k:
        k.run(d_xin, d_xout, d_projt, d_w1, d_w2e, d_b1c, d_lnw, d_lnb, d_out)

    nc.compile()
    return nc


class TileKernel:
    def __init__(self, nc, ln_trivial):
        self.nc = nc
        self.ln_trivial = ln_trivial
        self.ctx = ExitStack()

    def __enter__(self):
        self.tc = self.ctx.enter_context(tile.TileContext(self.nc))
        return self

    def __exit__(self, *a):
        return self.ctx.__exit__(*a)

    def pool(self, name, bufs, space="SBUF"):
        return self.ctx.enter_context(
            self.tc.tile_pool(name=name, bufs=bufs, space=space))

    # ================= top level =================
    def run(self, d_xin, d_xout, d_projt, d_w1, d_w2e, d_b1c, d_lnw, d_lnb,
            d_out):
        nc = self.nc

        const = self.pool("const", 1)
        wpool = self.pool("wts", 1)
        resid = self.pool("resid", 2)           # fp32 [128, 32*243] streams
        xbf = self.pool("xbf", 1)               # padded bf16 [128, 32*384]
        xt = self.pool("xt", 1)                 # transposed activations
        lth_p = self.pool("lth", 1)             # persistent l_in^T
        pqp = self.pool("pq", 3)                # pq^T j-major (per b,h)
        pkp = self.pool("pk", 2)                # pk token-major padded
        gel = self.pool("gelu", 10)
        st = self.pool("st", 4)                 # small stats tiles
        nbp = self.pool("nb", 2)                # -diag per head
        nbl = self.pool("nbL", 1)               # -diag of l_in final
        cxs = self.pool("cxs", 3)               # ctx_sb/ctxj/eps_sb

        ident = const.tile([128, 128], BF)
        masks.make_identity(nc, ident[:])
        zero_c = const.tile([128, 1], FP)
        nc.vector.memset(zero_c[:], 0.0)
        lneps_c = const.tile([128, 1], FP)
        nc.vector.memset(lneps_c[:], LNEPS)
        logeps_c = const.tile([128, 1], FP)
        nc.vector.memset(logeps_c[:], LOG_EPS)
        self.C = dict(ident=ident, zero=zero_c, lneps=lneps_c, logeps=logeps_c)
        if not self.ln_trivial:
            lnw_t = const.tile([128, D], FP)
            lnb_t = const.tile([128, D], FP)
            nc.sync.dma_start(out=lnw_t[:], in_=d_lnw)
            nc.sync.dma_start(out=lnb_t[:], in_=d_lnb)
            self.C["lnw"] = lnw_t
            self.C["lnb"] = lnb_t
        projt_t = []
        for a in range(6):
            # rows 0:81 = proj^T * dn, row 81 = 0 (skips the ones column of
            # the padded x^T), rows 82:84 = 1 (pick up the compensated -diag
            # rows so pk's exp needs no bias)
            t = const.tile([84, NF], BF, tag=f"projt{a}", name=f"projt{a}")
            nc.sync.dma_start(out=t[:], in_=d_projt[a])
            projt_t.append(t)

        self.P = dict(resid=resid, xbf=xbf, xt=xt, lth=lth_p, pq=pqp, pk=pkp,
                      gel=gel, st=st, nb=nbp, nbl=nbl, cxs=cxs, w=wpool)

        def load_ff_w(i):
            w1h = []
            for h in range(H):
                t = wpool.tile([F, FFH], BF, tag=f"w1h{h}", name=f"w1h{h}")
                nc.sync.dma_start(out=t[:], in_=d_w1[i, h * F:(h + 1) * F])
                w1h.append(t)
            w2k = []
            for kk in range(8):
                kw = 128 if kk < 7 else 97
                t = wpool.tile([kw, D], BF, tag=f"w2k{kk}", name=f"w2k{kk}")
                nc.sync.dma_start(out=t[:], in_=d_w2e[i, kk * 128: kk * 128 + kw])
                w2k.append(t)
            b1c = wpool.tile([128, 8], FP, tag="b1c", name="b1c")
            nc.sync.dma_start(out=b1c[:], in_=d_b1c[i])
            return w1h, w2k, b1c

        # ---------------- encoder ----------------
        X = resid.tile([128, NCH * YS], FP, tag="resid", name="resid")
        nc.sync.dma_start(out=X[:].rearrange("p (c d) -> p c d", d=YS),
                          in_=d_xin.rearrange("(c p) d -> p c d", p=128))
        Xb = self.make_bf16(X)
        nbh = self.diag_stats(X, self.P["nb"], raw=True)
        self.write_diag_cols(Xb, nbh)

        for i in range(NE):
            xth = self.transpose_heads(Xb)
            Y = resid.tile([128, NCH * YS], FP, tag="resid", name="resid")
            self.attention(projt_t[i], xth, nbh, Xb, X, Y)
            Yb = self.layer_norm(Y, bf16_out=True)[0]
            w1h, w2k, b1c = load_ff_w(i)
            X2 = resid.tile([128, NCH * YS], FP, tag="resid", name="resid")
            self.ff(Yb, Y, X2, w1h, w2k, b1c)
            last = (i == NE - 1)
            Xb, nbh = self.layer_norm(X2, bf16_out=True,
                                      diag_pool=(self.P["nbl"] if last
                                                 else self.P["nb"]))
            self.write_diag_cols(Xb, nbh)
            X = X2

        # persistent transposed l_in for the decoder cross attentions
        lth = [self.P["lth"].tile([84, NT], BF, tag=f"lth{h}", name=f"lth{h}")
               for h in range(H)]
        self.transpose_heads(Xb, dst=lth)
        nbhL = nbh

        # ---------------- decoder ----------------
        X = resid.tile([128, NCH * YS], FP, tag="resid", name="resid")
        nc.sync.dma_start(out=X[:].rearrange("p (c d) -> p c d", d=YS),
                          in_=d_xout.rearrange("(c p) d -> p c d", p=128))
        Xb = self.make_bf16(X)
        nbh = self.diag_stats(X, self.P["nb"], raw=True)
        self.write_diag_cols(Xb, nbh)

        for i in range(ND):
            xth = self.transpose_heads(Xb)
            Y = resid.tile([128, NCH * YS], FP, tag="resid", name="resid")
            self.attention(projt_t[2 + 2 * i], xth, nbh, Xb, X, Y)
            A1b = self.layer_norm(Y, bf16_out=True)[0]          # a1
            Y2 = resid.tile([128, NCH * YS], FP, tag="resid", name="resid")
            self.attention(projt_t[3 + 2 * i], lth, nbhL, A1b, Y, Y2)
            A2b = self.layer_norm(Y2, bf16_out=True)[0]         # a2
            w1h, w2k, b1c = load_ff_w(2 + i)
            X2 = resid.tile([128, NCH * YS], FP, tag="resid", name="resid")
            self.ff(A2b, Y2, X2, w1h, w2k, b1c)
            last = (i == ND - 1)
            if last:
                self.layer_norm(X2, bf16_out=False)
            else:
                Xb, nbh = self.layer_norm(X2, bf16_out=True,
                                          diag_pool=self.P["nb"])
                self.write_diag_cols(Xb, nbh)
            X = X2

        nc.sync.dma_start(out=d_out.rearrange("(c p) d -> p c d", p=128),
                          in_=X[:].rearrange("p (c d) -> p c d", d=YS))

    # ================= building blocks =================
    def _pad_out_view(self, Xb, g, c=None):
        """Padded output view [128, (4|1), 3, 81] for chunk group g or
        single chunk c."""
        if c is not None:
            return Xb[:, c * XS:(c + 1) * XS].rearrange(
                "p (h y) -> p h y", y=HS)[:, :, 0:F]
        return Xb[:, g * 4 * XS:(g + 1) * 4 * XS].rearrange(
            "p (c h y) -> p c h y", h=H, y=HS)[:, :, :, 0:F]

    def _flat_in_view(self, X, g):
        """4-chunk fp32 input view: [128, 4, 3, 81] at chunk group g."""
        return X[:, g * 4 * YS:(g + 1) * 4 * YS].rearrange(
            "p (c h f) -> p c h f", h=H, f=F)

    def make_bf16(self, X, Xb=None):
        """fp32 token-major -> padded bf16 (128-stride head slots, ones)."""
        nc = self.nc
        if Xb is None:
            Xb = self.P["xbf"].tile([128, NCH * XS], BF, tag="xbf", name="xbf")
        v3 = Xb[:].rearrange("p (c x) -> p c x", x=HS)
        nc.vector.memset(v3[:, :, F], 1.0)
        for g in range(NCH // 4):
            nc.any.tensor_copy(self._pad_out_view(Xb, g),
                               self._flat_in_view(X, g))
        return Xb

    def write_diag_cols(self, Xb, nbh):
        """Write compensated bf16(-diag) into head-slot cols 82/83 so the
        zk matmul contraction applies the diag bias exactly."""
        nc, st = self.nc, self.P["st"]
        v4 = Xb[:].rearrange("p (c h x) -> p c h x", h=H, x=HS)
        for h in range(H):
            co = st.tile([128, NCH], BF, tag="dco", name="dco")
            nc.vector.tensor_copy(co[:], nbh[h][:])
            re = st.tile([128, NCH], FP, tag="dre", name="dre")
            nc.vector.tensor_sub(re[:], nbh[h][:], co[:])
            nc.vector.tensor_copy(v4[:, :, h, 82], co[:])
            nc.vector.tensor_copy(v4[:, :, h, 83], re[:])

    def diag_stats(self, X, pool, raw, sw=None, mvg=None, rs2=None):
        """nbh[h] = -DSCALE * ||x_h||^2 (raw) or of the LN output (from
        even/odd bn_stats group algebra).  X used only when sw is None."""
        nc, st = self.nc, self.P["st"]
        if sw is None:
            sw = st.tile([128, 18 * NCH], FP, tag="sw", name="sw")
            for c in range(NCH):
                for h in range(H):
                    nc.vector.bn_stats(
                        out=sw[:, c * 18 + 6 * h: c * 18 + 6 * h + 6],
                        in_=X[:, c * YS + h * F: c * YS + (h + 1) * F])
        swv = sw[:].rearrange("p (c s) -> p c s", s=18)
        mu81 = None
        if not raw:
            mvv = mvg[:].rearrange("p (c t) -> p c t", t=2)
            mu = mvv[:, :, 0]
            mu81 = st.tile([128, NCH], FP, tag="mu81", name="mu81")
            nc.vector.tensor_scalar_mul(mu81[:], mu, float(D))
        nbh = [pool.tile([128, NCH], FP, tag=f"nbh{h}", name=f"nbh{h}")
               for h in range(H)]
        for h in range(H):
            me, m2e = swv[:, :, 6 * h + 1], swv[:, :, 6 * h + 2]
            mo, m2o = swv[:, :, 6 * h + 4], swv[:, :, 6 * h + 5]
            a = st.tile([128, NCH], FP, tag="dga", name="dga")
            q = st.tile([128, NCH], FP, tag="dgq", name="dgq")
            nc.vector.tensor_mul(a[:], me, me)
            nc.vector.scalar_tensor_tensor(out=q[:], in0=a[:], scalar=float(NEC),
                                           in1=m2e, op0=ALU.mult, op1=ALU.add)
            nc.vector.tensor_mul(a[:], mo, mo)
            nc.vector.scalar_tensor_tensor(out=a[:], in0=a[:], scalar=float(NOC),
                                           in1=m2o, op0=ALU.mult, op1=ALU.add)
            nc.vector.tensor_add(q[:], q[:], a[:])      # sum x^2 over head
            if raw:
                nc.vector.tensor_scalar_mul(nbh[h][:], q[:], -DSCALE)
                continue
            # s = 41*me + 40*mo ; u = (q + mu*(D*mu - 2*s)) * rs^2
            s = st.tile([128, NCH], FP, tag="dgs", name="dgs")
            nc.vector.tensor_scalar_mul(s[:], mo, float(NOC))
            nc.vector.scalar_tensor_tensor(out=s[:], in0=me, scalar=float(NEC),
                                           in1=s[:], op0=ALU.mult, op1=ALU.add)
            nc.vector.scalar_tensor_tensor(out=s[:], in0=s[:], scalar=-2.0,
                                           in1=mu81[:], op0=ALU.mult,
                                           op1=ALU.add)
            mvv = mvg[:].rearrange("p (c t) -> p c t", t=2)
            nc.vector.tensor_mul(s[:], s[:], mvv[:, :, 0])
            nc.vector.tensor_add(s[:], s[:], q[:])
            nc.vector.tensor_mul(s[:], s[:], rs2[:])
            nc.vector.tensor_scalar_mul(nbh[h][:], s[:], -DSCALE)
        return nbh

    def transpose_heads(self, Xb, dst=None):
        """padded token-major -> per-head feature-major via PE transposes.
        Transposes 84 slot columns: rows 0:81 features, row 81 the ones
        column (unused), rows 82:84 the compensated -diag rows."""
        nc = self.nc
        ident = self.C["ident"]
        if dst is None:
            dst = [self.P["xt"].tile([84, NT], BF, tag=f"xt{h}",
                                     name=f"xth{h}")
                   for h in range(H)]
        with ExitStack() as mctx:
            tp = mctx.enter_context(
                self.tc.tile_pool(name="tph", bufs=3, space="PSUM"))
            for c in range(NCH):
                for h in range(H):
                    ps = tp.tile([84, 128], BF, tag="tph", name="tph")
                    nc.tensor.transpose(
                        ps[0:84, 0:128],
                        Xb[:, c * XS + h * HS: c * XS + h * HS + 84],
                        ident[:, :])
                    nc.any.tensor_copy(dst[h][:, c * 128:(c + 1) * 128],
                                       ps[0:84, 0:128])
        return dst

    def attention(self, projt, xth, nbh, Vb, Xres, Y):
        """Y = performer_attention(q=k=xth-owner, v=Vb) + Xres.

        Batch-outer loop: each batch finishes all 3 heads before the next
        starts, so the consumer LayerNorm's per-chunk stats can overlap the
        remaining batches (keeps PE fed and the HAM warm)."""
        nc = self.nc
        ident = self.C["ident"]
        st, cxs = self.P["st"], self.P["cxs"]
        with ExitStack() as ps_ctx:
            zdp = ps_ctx.enter_context(
                self.tc.tile_pool(name="zdp", bufs=2, space="PSUM"))
            tpp = ps_ctx.enter_context(
                self.tc.tile_pool(name="tpp", bufs=2, space="PSUM"))
            ctxp = ps_ctx.enter_context(
                self.tc.tile_pool(name="ctxp", bufs=2, space="PSUM"))
            app = ps_ctx.enter_context(
                self.tc.tile_pool(name="app", bufs=2, space="PSUM"))
            drp = ps_ctx.enter_context(
                self.tc.tile_pool(name="drp", bufs=2, space="DRAM"))
            for b in range(NB):
                tb = b * L           # first token of the batch
                for h in range(H):
                    # ---- pq^T = exp(projt^T @ x^T), j-major, no bias ----
                    # pqT1 row 115 carries epsn (q-side eps rank-1 factor)
                    pqT0 = self.P["pq"].tile([128, L], BF, tag="pqT0",
                                             name="pqT0")
                    pqT1 = self.P["pq"].tile([116, L], BF, tag="pqT1",
                                             name="pqT1")
                    for jb, jw, pq in ((0, 128, pqT0), (1, 115, pqT1)):
                        for t in range(L // 512):
                            zt = zdp.tile([128, 512], FP, tag="zd", name="zd")
                            nc.tensor.matmul(
                                zt[0:jw, :],
                                projt[:, jb * 128: jb * 128 + jw],
                                xth[h][:, tb + t * 512: tb + (t + 1) * 512],
                                start=True, stop=True)
                            nc.scalar.activation(
                                pq[0:jw, t * 512:(t + 1) * 512],
                                zt[0:jw, :], ACTF.Exp,
                                bias=self.C["zero"][0:jw, :])
                    # ---- pk = exp(zd - diag), token-major; the -diag enters
                    # through the compensated rows 82:84 of xth/projt so the
                    # exp is bias-free and processes two chunks per op ----
                    pk = self.P["pk"].tile([128, CPB * KS], BF, tag="pk",
                                           name="pk")
                    nc.vector.memset(
                        pk[:].rearrange("p (c x) -> p c x", x=KS)[:, :, NF],
                        1.0)
                    for cp in range(CPB // 2):
                        zk2 = zdp.tile([128, 2 * NF], FP, tag="zd", name="zd")
                        for k in range(2):
                            c = b * CPB + 2 * cp + k
                            nc.tensor.matmul(
                                zk2[:, k * NF:(k + 1) * NF],
                                xth[h][:, c * 128:(c + 1) * 128],
                                projt[:, 0:NF], start=True, stop=True)
                        nc.scalar.activation(
                            pk[:, 2 * cp * KS: 2 * (cp + 1) * KS].rearrange(
                                "p (k x) -> p k x", x=KS)[:, :, 0:NF],
                            zk2[:].rearrange("p (k f) -> p k f", f=NF),
                            ACTF.Exp, bias=self.C["zero"][:])
                    nbs = nbh[h][:, b * CPB:(b + 1) * CPB]
                    mx = st.tile([128, CPB], FP, tag="mx", name="mx")
                    nc.vector.tensor_reduce(
                        mx[:],
                        pk[:].rearrange("p (c x) -> p c x", x=KS)[:, :, 0:NF],
                        axis=AX.X, op=ALU.max)
                    # ---- emx = mx*e^{diag} = e^{rm}; epsn = eps*emx (pq
                    # carries e^{-diag} too, so the eps factor simplifies) ----
                    ed = st.tile([128, CPB], FP, tag="ed", name="ed")
                    nc.scalar.activation(ed[:], nbs, ACTF.Exp, scale=-1.0,
                                         bias=self.C["zero"][:])
                    emx = st.tile([128, CPB], FP, tag="emx", name="emx")
                    nc.vector.tensor_mul(emx[:], mx[:], ed[:])
                    epsn = st.tile([128, CPB], BF, tag="epsn", name="epsn")
                    nc.vector.tensor_scalar_mul(epsn[:], emx[:], EPS)
                    # epsn -> pqT1 row 115 (transpose + DRAM bounce on the
                    # gpsimd SWDGE queue; matmul stationaries must start at
                    # partition 0/32/64 so a row tile is required)
                    tr = tpp.tile([CPB, 128], BF, tag="tp", name="tp")
                    nc.tensor.transpose(tr[0:CPB, 0:128], epsn[:, :],
                                        ident[:, :])
                    eps_sb = cxs.tile([CPB, 128], BF, tag="eps_sb",
                                      name="eps_sb")
                    nc.any.tensor_copy(eps_sb[:], tr[0:CPB, 0:128])
                    eps_d = drp.tile([CPB, 128], BF, tag="eps_d", name="eps_d")
                    nc.gpsimd.dma_start(out=eps_d[:], in_=eps_sb[:])
                    nc.gpsimd.dma_start(
                        out=pqT1[115:116, :].rearrange("p (c q) -> p c q",
                                                       q=128),
                        in_=eps_d[:].rearrange("(r c) q -> r c q", r=1))
                    # e^M over this batch's tokens
                    eMp = st.tile([128, 1], FP, tag="eMp", name="eMp")
                    nc.vector.tensor_reduce(eMp[:], emx[:], axis=AX.X,
                                            op=ALU.max)
                    eMa = st.tile([128, 1], FP, tag="eMa", name="eMa")
                    nc.gpsimd.partition_all_reduce(
                        eMa[:], eMp[:], channels=128,
                        reduce_op=bass_isa.ReduceOp.max)
                    # ctx^T [82, 244]: (v|1)^T @ (pk|1)
                    ctxT = ctxp.tile([82, 244], FP, tag="ctx", name="ctx")
                    for cc in range(CPB):
                        c = b * CPB + cc
                        nc.tensor.matmul(
                            ctxT[:],
                            Vb[:, c * XS + h * HS: c * XS + h * HS + 82],
                            pk[:, cc * KS: cc * KS + 244],
                            start=(cc == 0), stop=(cc == CPB - 1))
                    epsv = st.tile([82, 1], FP, tag="epsv", name="epsv")
                    nc.vector.tensor_scalar(out=epsv[:], in0=ctxT[:, 243:244],
                                            scalar1=eMa[0:82, :], scalar2=EPS,
                                            op0=ALU.mult, op1=ALU.mult)
                    # ctx_sb [82, 244]: cols 0:243 ctx + eps, col 243 = G
                    ctx_sb = cxs.tile([82, 244], BF, tag="ctx_sb",
                                      name="ctx_sb")
                    nc.vector.tensor_scalar(out=ctx_sb[:, 0:NF],
                                            in0=ctxT[:, 0:NF],
                                            scalar1=epsv[:], scalar2=None,
                                            op0=ALU.add)
                    gcol = st.tile([82, 1], FP, tag="gcol", name="gcol")
                    nc.vector.tensor_reduce(gcol[:], ctx_sb[:, 0:NF],
                                            axis=AX.X, op=ALU.add)
                    nc.any.tensor_copy(ctx_sb[:, 243:244], gcol[:])
                    # j-major ctx blocks; ctxj1 row 115 = G
                    t0 = tpp.tile([128, 82], BF, tag="tp", name="tp")
                    nc.tensor.transpose(t0[0:128, 0:82], ctx_sb[:, 0:128],
                                        ident[0:82, 0:82])
                    ctxj0 = cxs.tile([128, 82], BF, tag="ctxj0", name="ctxj0")
                    nc.any.tensor_copy(ctxj0[:], t0[0:128, 0:82])
                    t1 = tpp.tile([116, 82], BF, tag="tp", name="tp")
                    nc.tensor.transpose(t1[0:116, 0:82], ctx_sb[:, 128:244],
                                        ident[0:82, 0:82])
                    ctxj1 = cxs.tile([116, 82], BF, tag="ctxj1", name="ctxj1")
                    nc.any.tensor_copy(ctxj1[:], t1[0:116, 0:82])
                    # A = pq @ ctx + epsn x G ; out = A[:, :81]/A[:, 81] + res
                    # two chunks share one PSUM tile -> one strided recip
                    for cp in range(CPB // 2):
                        A = app.tile([128, 164], FP, tag="A", name="A")
                        for k in range(2):
                            cc = 2 * cp + k
                            csl = slice(cc * 128, (cc + 1) * 128)
                            asl = slice(k * 82, k * 82 + 82)
                            nc.tensor.matmul(A[:, asl], pqT0[:, csl],
                                             ctxj0[:], start=True, stop=False)
                            nc.tensor.matmul(A[:, asl], pqT1[:, csl],
                                             ctxj1[:], start=False, stop=True)
                        dinv = st.tile([128, 2], FP, tag="dinv", name="dinv")
                        nc.vector.reciprocal(
                            dinv[:],
                            A[:].rearrange("p (k e) -> p k e", e=82)[:, :, 81])
                        for k in range(2):
                            c = b * CPB + 2 * cp + k
                            ysl = Y[:, c * YS + h * F: c * YS + (h + 1) * F]
                            xsl = Xres[:, c * YS + h * F: c * YS + (h + 1) * F]
                            nc.vector.scalar_tensor_tensor(
                                out=ysl, in0=A[:, k * 82: k * 82 + 81],
                                scalar=dinv[:, k:k + 1], in1=xsl,
                                op0=ALU.mult, op1=ALU.add)

    def layer_norm(self, Y, bf16_out, diag_pool=None):
        """In-place LN on Y.  Returns (padded bf16 tile or None, nbh or None)."""
        nc, st = self.nc, self.P["st"]
        need_diag = diag_pool is not None
        per_head = need_diag and self.ln_trivial
        sw = st.tile([128, 18 * NCH], FP, tag="sw", name="sw")
        mvg = st.tile([128, 2 * NCH], FP, tag="mvg", name="mvg")
        for c in range(NCH):
            if per_head:
                for h in range(H):
                    nc.vector.bn_stats(
                        out=sw[:, c * 18 + 6 * h: c * 18 + 6 * h + 6],
                        in_=Y[:, c * YS + h * F: c * YS + (h + 1) * F])
                nc.vector.bn_aggr(out=mvg[:, 2 * c:2 * c + 2],
                                  in_=sw[:, c * 18:(c + 1) * 18])
            else:
                nc.vector.bn_stats(out=sw[:, c * 18: c * 18 + 6],
                                   in_=Y[:, c * YS:(c + 1) * YS])
                nc.vector.bn_aggr(out=mvg[:, 2 * c:2 * c + 2],
                                  in_=sw[:, c * 18: c * 18 + 6])
        mvv = mvg[:].rearrange("p (c t) -> p c t", t=2)
        mu, var = mvv[:, :, 0], mvv[:, :, 1]
        sd = st.tile([128, NCH], FP, tag="lnsd", name="lnsd")
        nc.scalar.activation(sd[:], var, ACTF.Sqrt, bias=self.C["lneps"][:])
        rs = st.tile([128, NCH], FP, tag="lnrs", name="lnrs")
        nc.vector.reciprocal(rs[:], sd[:])
        nmr = st.tile([128, NCH], FP, tag="lnnmr", name="lnnmr")
        nc.vector.scalar_tensor_tensor(out=nmr[:], in0=mu, scalar=-1.0,
                                       in1=rs[:], op0=ALU.mult, op1=ALU.mult)
        nbh = None
        if need_diag and self.ln_trivial:
            rs2 = st.tile([128, NCH], FP, tag="lnrs2", name="lnrs2")
            nc.vector.tensor_mul(rs2[:], rs[:], rs[:])
            nbh = self.diag_stats(None, diag_pool, raw=False, sw=sw, mvg=mvg,
                                  rs2=rs2)
        Xb = None
        if bf16_out:
            Xb = self.P["xbf"].tile([128, NCH * XS], BF, tag="xbf", name="xbf")
            v3 = Xb[:].rearrange("p (c x) -> p c x", x=HS)
            nc.vector.memset(v3[:, :, F], 1.0)
        for c in range(NCH):
            sl = Y[:, c * YS:(c + 1) * YS]
            if bf16_out and self.ln_trivial:
                # bf16 apply straight from raw Y on the scalar engine,
                # independent of (and parallel to) the fp32 in-place apply
                nc.scalar.activation(
                    self._pad_out_view(Xb, None, c),
                    sl.rearrange("p (h f) -> p h f", f=F),
                    ACTF.Identity, bias=nmr[:, c:c + 1],
                    scale=rs[:, c:c + 1])
            nc.vector.tensor_scalar(out=sl, in0=sl, scalar1=rs[:, c:c + 1],
                                    scalar2=nmr[:, c:c + 1],
                                    op0=ALU.mult, op1=ALU.add)
            if not self.ln_trivial:
                nc.vector.tensor_mul(sl, sl, self.C["lnw"][:])
                nc.vector.tensor_add(sl, sl, self.C["lnb"][:])
        if bf16_out and not self.ln_trivial:
            for g in range(NCH // 4):
                nc.any.tensor_copy(self._pad_out_view(Xb, g),
                                   self._flat_in_view(Y, g))
        if need_diag and not self.ln_trivial:
            nbh = [diag_pool.tile([128, NCH], FP, tag=f"nbh{h}", name=f"nbh{h}")
                   for h in range(H)]
            for h in range(H):
                for c in range(NCH):
                    sl = Y[:, c * YS + h * F: c * YS + (h + 1) * F]
                    s = st.tile([128, F], FP, tag="sqh", name="sqh")
                    nc.vector.tensor_mul(s[:], sl, sl)
                    nc.vector.tensor_reduce(nbh[h][:, c:c + 1], s[:],
                                            axis=AX.X, op=ALU.add)
                nc.vector.tensor_scalar_mul(nbh[h][:], nbh[h][:], -DSCALE)
        return Xb, nbh

    def ff(self, Yb, FFIN, Ynew, w1h, w2k, b1c):
        """Ynew = gelu(FFIN@w1+b1)@w2 + b2 + FFIN."""
        nc = self.nc
        with ExitStack() as ps_ctx:
            f1p = ps_ctx.enter_context(
                self.tc.tile_pool(name="f1p", bufs=3, space="PSUM"))
            f2p = ps_ctx.enter_context(
                self.tc.tile_pool(name="f2p", bufs=2, space="PSUM"))
            # per-head feature-major DMA transposes (tags shared with the
            # attention head-transposes; never live at the same time)
            fth = self.transpose_heads(Yb)
            for ng in range(NT // 512):
                gts = []
                for kk in range(8):
                    mw = 128 if kk < 7 else 76
                    f1 = f1p.tile([128, 512], FP, tag="f1", name="f1")
                    for h in range(H):
                        nc.tensor.matmul(f1[0:mw, :],
                                         w1h[h][:, kk * 128: kk * 128 + mw],
                                         fth[h][0:F, ng * 512:(ng + 1) * 512],
                                         start=(h == 0), stop=(h == H - 1))
                    gt = self.P["gel"].tile([128, 512], BF, tag="g", name="g")
                    if kk == 7:
                        nc.vector.memset(gt[64:128, :], 0.0)
                    nc.scalar.activation(gt[0:mw, :], f1[0:mw, :], ACTF.Gelu,
                                         bias=b1c[0:mw, kk:kk + 1])
                    if kk == 7:
                        nc.vector.memset(gt[96:97, :], 1.0)
                    gts.append(gt)
                for j in range(4):
                    c = ng * 4 + j
                    f2 = f2p.tile([128, D], FP, tag="f2", name="f2")
                    for kk in range(8):
                        kw = 128 if kk < 7 else 97
                        nc.tensor.matmul(f2[:],
                                         gts[kk][0:kw, j * 128:(j + 1) * 128],
                                         w2k[kk][:],
                                         start=(kk == 0), stop=(kk == 7))
                    nc.vector.tensor_add(Ynew[:, c * YS:(c + 1) * YS], f2[:],
                                         FFIN[:, c * YS:(c + 1) * YS])


# ---------------- host side ----------------
def _prep_inputs(patches, ln_w, ln_b, enc_proj, enc_w1, enc_b1, enc_w2, enc_b2,
                 dec1_proj, dec2_proj, dec_w1, dec_b1, dec_w2, dec_b2):
    bf = ml_dtypes.bfloat16
    p = np.ascontiguousarray(patches).reshape(L, 2, B, D)
    projs = [enc_proj[0], enc_proj[1], dec1_proj[0], dec2_proj[0],
             dec1_proj[1], dec2_proj[1]]
    projt = np.zeros((6, 84, NF), np.float32)
    for a, pr in enumerate(projs):
        projt[a, :F] = np.asarray(pr).T * DN
        projt[a, 82:84] = 1.0
    projt = projt.astype(bf)
    w1s = np.stack([enc_w1[0], enc_w1[1], dec_w1[0], dec_w1[1]]).astype(bf)
    w2e = np.zeros((4, 993, D), np.float32)
    b1c = np.zeros((4, 128, 8), np.float32)
    for i, (w2, b1, b2) in enumerate([
            (enc_w2[0], enc_b1[0], enc_b2[0]), (enc_w2[1], enc_b1[1], enc_b2[1]),
            (dec_w2[0], dec_b1[0], dec_b2[0]), (dec_w2[1], dec_b1[1], dec_b2[1])]):
        w2e[i, :FFH] = np.asarray(w2)
        w2e[i, 992] = np.asarray(b2)
        b1p = np.zeros(1024, np.float32)
        b1p[:FFH] = np.asarray(b1)
        b1c[i] = b1p.reshape(8, 128).T
    w2e = w2e.astype(bf)
    lnw = np.tile(np.asarray(ln_w, np.float32)[None, :], (128, 1))
    lnb = np.tile(np.asarray(ln_b, np.float32)[None, :], (128, 1))
    ln_trivial = bool(np.all(np.asarray(ln_w) == 1.0)
                      and np.all(np.asarray(ln_b) == 0.0))
    in_maps = []
    for c in range(NCORES):
        bs = slice(NB * c, NB * (c + 1))
        xin = np.ascontiguousarray(
            p[:, 0, bs].transpose(1, 0, 2).reshape(NT, D)).astype(np.float32)
        xout = np.ascontiguousarray(
            p[:, 1, bs].transpose(1, 0, 2).reshape(NT, D)).astype(np.float32)
        in_maps.append(dict(xin=xin, xout=xout, projt=projt, w1=w1s, w2e=w2e,
                            b1c=b1c, lnw=lnw, lnb=lnb))
    return in_maps, ln_trivial


def kernel(**inputs):
    in_maps, ln_trivial = _prep_inputs(**inputs)
    key = ("nc", ln_trivial)
    if key not in _cache:
        _cache[key] = _build(ln_trivial)
    nc = _cache[key]
    res = run_bass_kernel_spmd(
        nc, in_maps, list(range(NCORES)),
        trace=bool(int(os.environ.get("KERNEL_TRACE", "0"))))
    kernel._last_result = res
    out = np.concatenate([res.results[c]["out"] for c in range(NCORES)], axis=0)
    return out
